# revision 1
# baseline (speedup 1.0000x reference)
"""Distributed 3-layer GAT kernel for TRN2 (8 NeuronCores), v2.

Node layout: nodes greedy-packed into NBINS = NCORES*TPC bins of <=128 slots,
balanced by in-degree. Global row of node n = bin*128 + slot; core c owns bins
[c*TPC,(c+1)*TPC) = rows [c*NPC,(c+1)*NPC).

Per layer:
  A-step : xh[, a_s, a_d] = h @ [lin | att folds]  (TensorE); scores kept as
           raw f32 bytes in bf16 cols [HC, HC+16) via bitcast. DMA to xh_loc,
           grouped AllGather -> xh_full [NSLOT, RW] bf16.
  C-step : per dst tile t (double-buffered): ONE whole-tile dma_gather of src
           rows (RW wide, scores ride along) + ONE dst-score gather (SCW wide).
           Batched score math -> alpha; masks built on-chip via
           is_equal(iota, dslot); denominator via mask lhsT matmuls; alpha
           folded into G rows in-place; 2 message matmuls of 512 cols per
           chunk accumulate in PSUM; epilogue *1/denom, +bias, ELU; T-step
           (transpose to hT) interleaved per tile.
"""
import sys
sys.path.insert(0, "/opt/trn_rl_repo")
import os
from dataclasses import dataclass

import numpy as np
import ml_dtypes

import concourse.bass as bass
import concourse.bacc as bacc
import concourse.mybir as mybir
from concourse.library_config import mlp

BF16 = mybir.dt.bfloat16
F32 = mybir.dt.float32
I16 = mybir.dt.int16
ALU = mybir.AluOpType
ACT = mybir.ActivationFunctionType


@dataclass
class Cfg:
    NCORES: int = 8
    TPC: int = 10
    H: int = 4
    HID: int = 256
    D: int = 384
    OUT: int = 128
    KMAX: int = 17

    @property
    def HC(self):
        return self.H * self.HID

    @property
    def SCW(self):
        return 128

    @property
    def RW(self):
        return self.HC + self.SCW

    @property
    def NBINS(self):
        return self.NCORES * self.TPC

    @property
    def NSLOT(self):
        return self.NBINS * 128

    @property
    def NPC(self):
        return self.TPC * 128

    @property
    def DIN(self):
        return self.D + self.HID


def cdiv(a, b):
    return (a + b - 1) // b


def build_graph(cfg: Cfg):
    PHASES = int(os.environ.get("GAT_PHASES", "4"))
    H, HID, HC, RW, SCW = cfg.H, cfg.HID, cfg.HC, cfg.RW, cfg.SCW
    TPC, KMAX, NPC, NSLOT, OUT = cfg.TPC, cfg.KMAX, cfg.NPC, cfg.NSLOT, cfg.OUT
    DINP = cdiv(cfg.DIN, 128)
    HCP = HC // 128
    HIDP = HID // 128
    ACOLS = HC + 8
    assert cfg.DIN % 128 == 0 and HC % 128 == 0 and HID % 128 == 0

    nc = bacc.Bacc("TRN2")

    p_inaug = nc.declare_dram_parameter("in_augT", [DINP, 128, NPC], BF16, isOutput=False)
    p_win = nc.declare_dram_parameter("w_in", [DINP, 128, HID], BF16, isOutput=False)
    p_binT = nc.declare_dram_parameter("b_inT", [128, HIDP], F32, isOutput=False)
    p_lin = [nc.declare_dram_parameter(f"lin{l}", [HIDP if l == 0 else HCP, 128, ACOLS], BF16, isOutput=False) for l in range(3)]
    p_bias = [nc.declare_dram_parameter(f"bias_bc{l}", [128, HC], F32, isOutput=False) for l in range(3)]
    p_wout = nc.declare_dram_parameter("w_out", [HCP, 128, OUT], BF16, isOutput=False)
    p_boutT = nc.declare_dram_parameter("b_outT", [128, 1], F32, isOutput=False)
    p_ident = nc.declare_dram_parameter("ident", [128, 128], BF16, isOutput=False)
    p_isrc = nc.declare_dram_parameter("idx_src", [128, TPC * KMAX * 8], I16, isOutput=False)
    p_idst = nc.declare_dram_parameter("idx_dst", [128, TPC * KMAX * 8], I16, isOutput=False)
    p_dslot = nc.declare_dram_parameter("dslot", [128, TPC * KMAX], F32, isOutput=False)
    p_iota = nc.declare_dram_parameter("iota_bc", [128, 128], BF16, isOutput=False)
    p_out = nc.declare_dram_parameter("outT", [128, NPC], F32, isOutput=True)

    xh_loc = nc.dram_tensor("xh_loc", [NPC, RW], BF16)
    xh_full2 = [nc.dram_tensor(f"xh_full{i}", [NSLOT, RW], BF16, addr_space="Shared")
                for i in range(2)]

    from contextlib import ExitStack
    st = ExitStack()
    with st:
        sb_inaug = st.enter_context(nc.sbuf_tensor("sb_inaug", [128, DINP, NPC], BF16))
        sb_win = st.enter_context(nc.sbuf_tensor("sb_win", [128, DINP, HID], BF16))
        sb_binT = st.enter_context(nc.sbuf_tensor("sb_binT", [128, HIDP], F32))
        sb_lin = st.enter_context(nc.sbuf_tensor("sb_lin", [128, HCP, ACOLS], BF16))
        sb_bias = st.enter_context(nc.sbuf_tensor("sb_bias", [128, 3, HC], F32))
        sb_wout = st.enter_context(nc.sbuf_tensor("sb_wout", [128, HCP, OUT], BF16))
        sb_boutT = st.enter_context(nc.sbuf_tensor("sb_boutT", [128, 1], F32))
        sb_ident = st.enter_context(nc.sbuf_tensor("sb_ident", [128, 128], BF16))
        sb_isrc = st.enter_context(nc.sbuf_tensor("sb_isrc", [128, TPC * KMAX * 8], I16))
        sb_idst = st.enter_context(nc.sbuf_tensor("sb_idst", [128, TPC * KMAX * 8], I16))
        sb_dslot = st.enter_context(nc.sbuf_tensor("sb_dslot", [128, TPC * KMAX], F32))
        sb_iota = st.enter_context(nc.sbuf_tensor("sb_iota", [128, 128], BF16))
        sb_hT = st.enter_context(nc.sbuf_tensor("sb_hT", [128, HCP, NPC], BF16))
        sb_hnm = st.enter_context(nc.sbuf_tensor("sb_hnm", [128, 2, HC], BF16))
        sb_stage = st.enter_context(nc.sbuf_tensor("sb_stage", [128, 2, RW], BF16))
        sb_Gt = st.enter_context(nc.sbuf_tensor("sb_Gt", [128, 2, KMAX, RW], BF16))
        sb_Sd = st.enter_context(nc.sbuf_tensor("sb_Sd", [128, 2, KMAX, SCW], BF16))
        sb_msk = st.enter_context(nc.sbuf_tensor("sb_msk", [128, 2, KMAX, 128], BF16))
        sb_W4 = st.enter_context(nc.sbuf_tensor("sb_W4", [128, 2, 4, 128], BF16))
        sb_alf = st.enter_context(nc.sbuf_tensor("sb_alf", [128, 2, KMAX, 4], F32))
        sb_alb = st.enter_context(nc.sbuf_tensor("sb_alb", [128, 2, KMAX, 4], BF16))
        sb_sc1 = st.enter_context(nc.sbuf_tensor("sb_sc1", [128, KMAX, 4], F32))
        sb_sc2 = st.enter_context(nc.sbuf_tensor("sb_sc2", [128, KMAX, 4], F32))
        sb_sc3 = st.enter_context(nc.sbuf_tensor("sb_sc3", [128, KMAX, 4], F32))
        sb_tmp4 = st.enter_context(nc.sbuf_tensor("sb_tmp4", [128, 4], F32))
        sb_rden = st.enter_context(nc.sbuf_tensor("sb_rden", [128, 2, 4], F32))
        sb_ep1 = st.enter_context(nc.sbuf_tensor("sb_ep1", [128, HC], F32))
        sb_ep2 = st.enter_context(nc.sbuf_tensor("sb_ep2", [128, HC], F32))
        sb_ep3 = st.enter_context(nc.sbuf_tensor("sb_ep3", [128, HC], F32))
        sb_osb = st.enter_context(nc.sbuf_tensor("sb_osb", [128, NPC], F32))
        pb = [st.enter_context(nc.psum_tensor(f"pb{i}", [128, 512], F32)) for i in range(4)]
        pbT = [st.enter_context(nc.psum_tensor(f"pbT{i}", [128, 128], F32)) for i in range(2)]
        ps_a3 = st.enter_context(nc.psum_tensor("ps_a3", [128, 8], F32))
        ps_den = st.enter_context(nc.psum_tensor("ps_den", [128, 4], F32))
        s_pdma = st.enter_context(nc.semaphore("pdma"))
        s_gthA = st.enter_context(nc.semaphore("gthA"))
        s_gthB = st.enter_context(nc.semaphore("gthB"))
        s_gthC = st.enter_context(nc.semaphore("gthC"))
        s_gthD = st.enter_context(nc.semaphore("gthD"))
        s_xdA = st.enter_context(nc.semaphore("xdA"))
        s_xdB = st.enter_context(nc.semaphore("xdB"))
        s_cc = st.enter_context(nc.semaphore("cc"))
        s_mm = st.enter_context(nc.semaphore("mm"))
        s_vv = st.enter_context(nc.semaphore("vv"))
        s_aa = st.enter_context(nc.semaphore("aa"))
        block = st.enter_context(nc.Block())
        sem = {"pdma": s_pdma, "gthA": s_gthA, "gthB": s_gthB,
               "gthC": s_gthC, "gthD": s_gthD,
               "xdA": s_xdA, "xdB": s_xdB, "cc": s_cc,
               "mm": s_mm, "vv": s_vv, "aa": s_aa}
        prog = {"g": [], "t": [], "v": [], "a": [], "s": []}
        cnt = {k: 0 for k in sem}
        reg_cache = {}

        def nreg(e, v):
            key = (id(e), v)
            if key not in reg_cache:
                reg_cache[key] = e.to_reg(v)
            return reg_cache[key]

        def op(eng, fn, inc=None, amt=1):
            prog[eng].append(("op", fn, inc, amt))
            if inc:
                cnt[inc] += amt
                return cnt[inc]
            return None

        def wt(eng, sm, val):
            if val and val > 0:
                prog[eng].append(("wait", sm, val))

        # ============ phase 0: loads ============
        loads = [
            (sb_inaug[:], bass.AP(p_inaug, 0, [[NPC, 128], [128 * NPC, DINP], [1, NPC]])),
            (sb_win[:], bass.AP(p_win, 0, [[HID, 128], [128 * HID, DINP], [1, HID]])),
            (sb_binT[:], p_binT[:]),
            (sb_bias[:, 0, :], p_bias[0][:]),
            (sb_bias[:, 1, :], p_bias[1][:]),
            (sb_bias[:, 2, :], p_bias[2][:]),
            (sb_wout[:], bass.AP(p_wout, 0, [[OUT, 128], [128 * OUT, HCP], [1, OUT]])),
            (sb_boutT[:], p_boutT[:]),
            (sb_ident[:], p_ident[:]),
            (sb_isrc[:], p_isrc[:]),
            (sb_idst[:], p_idst[:]),
            (sb_dslot[:], p_dslot[:]),
            (sb_iota[:], p_iota[:]),
            (sb_lin[:, 0:HIDP, :], bass.AP(p_lin[0], 0, [[ACOLS, 128], [128 * ACOLS, HIDP], [1, ACOLS]])),
        ]
        for d, sr in loads:
            op("s", lambda e, d=d, sr=sr: e.dma_start(d, sr), "pdma", 16)
        pdma_loaded = cnt["pdma"]
        op("v", lambda e: e.memset(sb_stage[:, :, HC + 16:RW], 0), "vv")
        if PHASES < 4:
            op("v", lambda e: e.memset(sb_hT[:], 0), "vv")
            op("v", lambda e: e.memset(sb_hnm[:], 0), "vv")
            op("v", lambda e: e.drain())
        for eng in ("g", "t", "v", "a"):
            wt(eng, "pdma", pdma_loaded)

        # persistent cross-step state
        psum_free_vv = {0: 0, 1: 0, 2: 0, 3: 0}   # pb free-after vv
        pbT_free_vv = {0: 0, 1: 0}
        psa3_free = [0]
        pden_free_vv = 0
        stage_free_xdma = {0: None, 1: None}
        Gt_free_mm = {0: 0, 1: 0}
        W4_free_state = {0: 0, 1: 0}
        Sd_free_vv = {0: 0, 1: 0}
        msk_free_mm = {0: 0, 1: 0}
        alf_free = {0: (0, 0), 1: (0, 0)}   # (vv, aa) after scales of that buf
        rden_free_vv = {0: 0, 1: 0}
        hnm_free_mm = {0: 0, 1: 0}
        hT_ready_vv = 0

        REPS = int(os.environ.get("GAT_REPS", "1"))
        AGS = int(os.environ.get("GAT_AGSPLIT", "5"))
        GSPLIT = int(os.environ.get("GAT_GSPLIT", "8"))
        GREP = int(os.environ.get("GAT_GREP", "1"))
        SREP = int(os.environ.get("GAT_SREP", "1"))
        CCREP = int(os.environ.get("GAT_CCREP", "1"))
        NLAYERS = 3 if PHASES >= 4 else min(PHASES, 1)
        do_C = PHASES >= 2
        do_T = PHASES >= 3
        NIDX = KMAX * 128
        NF = [(i * 512, min((i + 1) * 512, NPC)) for i in range(cdiv(NPC, 512))]
        gsz = (TPC + AGS - 1) // AGS
        NCR = cfg.NCORES * 128

        pdma_lin = {0: pdma_loaded}
        ccv_layer = {}
        hTcol_vv = {}   # per (l, tile): hT cols ready after T-copies

        def rec_lin_reload(l):
            wt("s", "mm", cnt["mm"])
            op("s", lambda e, l=l: e.dma_start(
                sb_lin[:, 0:HCP, :],
                bass.AP(p_lin[l], 0, [[ACOLS, 128], [128 * ACOLS, HCP], [1, ACOLS]])), "pdma", 16)
            pdma_lin[l] = cnt["pdma"]

        def rec_A_bin(l, b, hT_gate):
            inch_p = HIDP if l == 0 else HCP
            wt("t", "pdma", pdma_lin[l])
            wt("t", "vv", hT_gate)
            fch = [(0, 512, pb[2], 2), (512, 1024, pb[3], 3), (1024, ACOLS, ps_a3, -1)]
            mmv = None
            for (f0, f1, pst, slot) in fch:
                if slot < 0:
                    wt("t", "vv", psa3_free[0])
                else:
                    wt("t", "vv", psum_free_vv[slot])
                psl = pst[:, 0:f1 - f0]
                for c in range(inch_p):
                    mmv = op("t", lambda e, psl=psl, c=c, b=b, f0=f0, f1=f1, inch_p=inch_p:
                             e.matmul(psl, sb_hT[:, c, b * 128:(b + 1) * 128],
                                      sb_lin[:, c, f0:f1],
                                      start=(c == 0), stop=(c == inch_p - 1)),
                             "mm" if c == inch_p - 1 else None)
            sslot = b % 2
            wt("v", "pdma", pdma_lin[l])
            wt("v", "mm", mmv)
            if stage_free_xdma[sslot]:
                wt("v", *stage_free_xdma[sslot])
            op("v", lambda e, sslot=sslot: e.tensor_copy(sb_stage[:, sslot, 0:512], pb[2][:]))
            op("v", lambda e, sslot=sslot: e.tensor_copy(sb_stage[:, sslot, 512:1024], pb[3][:]))
            vvv = op("v", lambda e, sslot=sslot: e.tensor_copy(
                sb_stage[:, sslot, HC:HC + 16].bitcast(F32), ps_a3[:]), "vv")
            psum_free_vv[2] = psum_free_vv[3] = psa3_free[0] = vvv
            wt("s", "vv", vvv)
            xsem = "xdA" if sslot == 0 else "xdB"
            xdv = op("s", lambda e, b=b, sslot=sslot:
                     e.dma_start(xh_loc[b * 128:(b + 1) * 128, :], sb_stage[:, sslot, :]),
                     xsem, 16)
            stage_free_xdma[sslot] = (xsem, xdv)
            if (b + 1) % gsz == 0 or b == TPC - 1:
                b0g = (b // gsz) * gsz
                nbg = b - b0g + 1
                wt("g", xsem, xdv)
                if b > 0 and stage_free_xdma[1 - sslot]:
                    wt("g", *stage_free_xdma[1 - sslot])
                xf = xh_full2[l % 2]
                for _cc in range(CCREP):
                    ccv_layer[l] = op("g", lambda e, b0g=b0g, nbg=nbg, xf=xf: e.collective_compute(
                        "AllGather", ALU.bypass,
                        replica_groups=[list(range(cfg.NCORES))],
                        ins=[xh_loc[b0g * 128:(b0g + nbg) * 128, :]],
                        outs=[xf[b0g * NCR:(b0g + nbg) * NCR, :]]), "cc", 1)

        for rep in range(REPS):
            if rep > 0:
                wt("s", "mm", cnt["mm"])
                wt("s", "vv", cnt["vv"])
                op("s", lambda e: e.dma_start(
                    sb_lin[:, 0:HIDP, :],
                    bass.AP(p_lin[0], 0, [[ACOLS, 128], [128 * ACOLS, HIDP], [1, ACOLS]])), "pdma", 16)
                pdma_lin[0] = cnt["pdma"]
                wt("t", "pdma", pdma_lin[0])

            # ============ IN-step: hT[:, 0:HIDP, :] = (w_in.T @ in_aug) + b_in ============
            grp = 0
            for oc in range(HIDP):
                for (f0, f1) in NF:
                    slot = grp % 2
                    psl = pb[slot][:, 0:f1 - f0]
                    wt("t", "vv", psum_free_vv[slot])
                    for c in range(DINP):
                        mmv = op("t", lambda e, psl=psl, c=c, oc=oc, f0=f0, f1=f1:
                                 e.matmul(psl, sb_win[:, c, oc * 128:(oc + 1) * 128],
                                          sb_inaug[:, c, f0:f1],
                                          start=(c == 0), stop=(c == DINP - 1)),
                                 "mm" if c == DINP - 1 else None)
                    wt("v", "mm", mmv)
                    vvv = op("v", lambda e, psl=psl, oc=oc, f0=f0, f1=f1:
                             e.tensor_scalar(sb_hT[:, oc, f0:f1], psl,
                                             sb_binT[:, oc:oc + 1], None, ALU.add), "vv")
                    psum_free_vv[slot] = vvv
                    grp += 1
            hT_ready_vv = cnt["vv"]

            # ============ prologue: A(0) + AG(0) ============
            for b in range(TPC):
                rec_A_bin(0, b, hT_ready_vv)
            if NLAYERS > 1:
                rec_lin_reload(1)

            # ============ layers (C with interleaved T, A(l+1), AG(l+1)) ============
            for l in range(NLAYERS):
                if not do_C:
                    break
                wt("g", "cc", ccv_layer[l])
                xf = xh_full2[l % 2]
                xh_g = bass.AP(xf, 0, [[RW, NSLOT], [1, RW]])
                xh_sc = bass.AP(xf, HC, [[RW, NSLOT], [1, SCW]])
                gtv = {}
                sdv = {}
                den_mm = {}
                msg_mm = {}
                hnm_ready = {}

                def rec_gather(t, xh_g=None, xh_sc=None):
                    buf = t % 2
                    icol = (t * KMAX) * 8
                    gsm = "gthA" if buf == 0 else "gthC"
                    ssm = "gthB" if buf == 0 else "gthD"
                    wt("g", "mm", Gt_free_mm[buf])
                    wt("g", gsm, gtv.get(t - 2, 0))
                    for _gr in range(GREP):
                        for k0 in range(0, KMAX, GSPLIT):
                            nch = min(GSPLIT, KMAX - k0)
                            gtv[t] = op("g", lambda e, icol=icol, buf=buf, k0=k0, nch=nch, xh_g=xh_g: e.dma_gather(
                                sb_Gt[:, buf, k0:k0 + nch, :], xh_g,
                                sb_isrc[:, icol + k0 * 8:icol + (k0 + nch) * 8],
                                nch * 128, nreg(e, nch * 128), RW, elem_step=RW), gsm, 16)
                    wt("g", "vv", Sd_free_vv[buf])
                    wt("g", ssm, sdv.get(t - 2, 0))
                    for _sr in range(SREP):
                        for k0 in range(0, KMAX, GSPLIT):
                            nch = min(GSPLIT, KMAX - k0)
                            sdv[t] = op("g", lambda e, icol=icol, buf=buf, k0=k0, nch=nch, xh_sc=xh_sc: e.dma_gather(
                                sb_Sd[:, buf, k0:k0 + nch, :], xh_sc,
                                sb_idst[:, icol + k0 * 8:icol + (k0 + nch) * 8],
                                nch * 128, nreg(e, nch * 128), SCW, elem_step=RW), ssm, 16)

                rec_gather(0, xh_g=xh_g, xh_sc=xh_sc)
                rec_gather(1, xh_g=xh_g, xh_sc=xh_sc)

                for t in range(TPC):
                    buf = t % 2
                    # --- masks for this tile (v) ---
                    wt("v", "mm", msk_free_mm[buf])
                    mskv = None
                    for k in range(KMAX):
                        mskv = op("v", lambda e, t=t, k=k, buf=buf: e.tensor_scalar(
                            sb_msk[:, buf, k, :], sb_iota[:],
                            sb_dslot[:, t * KMAX + k:t * KMAX + k + 1], None, ALU.is_equal),
                            "vv" if k == KMAX - 1 else None)
                    msk_ready = mskv
                    # --- score math (v + a) ---
                    wt("v", "gthA" if buf == 0 else "gthC", gtv[t])
                    wt("v", "gthB" if buf == 0 else "gthD", sdv[t])
                    av, aa_ = alf_free[buf]
                    wt("v", "vv", av)
                    wt("v", "aa", aa_)
                    GtF = sb_Gt[:, buf, :, :].bitcast(F32)   # [128, KMAX, RW//2]
                    SdF = sb_Sd[:, buf, :, :].bitcast(F32)   # [128, KMAX, SCW//2]
                    op("v", lambda e, GtF=GtF, SdF=SdF: e.tensor_add(
                        sb_sc1[:], GtF[:, :, HC // 2:HC // 2 + 4], SdF[:, :, 4:8]))
                    op("v", lambda e: e.drain())
                    op("v", lambda e: e.tensor_scalar(sb_sc2[:], sb_sc1[:], 0.0, None, ALU.max))
                    op("v", lambda e: e.tensor_scalar(sb_sc3[:], sb_sc1[:], 0.0, 0.2, ALU.min, ALU.mult))
                    op("v", lambda e: e.drain())
                    vvv = op("v", lambda e: e.tensor_add(sb_sc1[:], sb_sc2[:], sb_sc3[:]), "vv")
                    wt("a", "vv", vvv)
                    aav = op("a", lambda e, buf=buf: e.activation(
                        sb_alf[:, buf, :, :], sb_sc1[:], ACT.Exp), "aa")
                    wt("v", "aa", aav)
                    vvv = op("v", lambda e, buf=buf: e.tensor_copy(
                        sb_alb[:, buf, :, :], sb_alf[:, buf, :, :]), "vv")
                    alb_ready = vvv
                    Sd_free_vv[buf] = vvv

                    # --- denominator (t) ---
                    wt("t", "vv", alb_ready)
                    wt("t", "vv", msk_ready)
                    wt("t", "vv", pden_free_vv)
                    mmv = None
                    for k in range(KMAX):
                        mmv = op("t", lambda e, k=k, buf=buf: e.matmul(
                            ps_den[:], sb_msk[:, buf, k, :], sb_alb[:, buf, k, :],
                            start=(k == 0), stop=(k == KMAX - 1)),
                            "mm" if k == KMAX - 1 else None)
                    den_mm[t] = mmv
                    # --- reciprocal (v) ---
                    wt("v", "mm", den_mm[t])
                    wt("v", "vv", rden_free_vv[buf])
                    op("v", lambda e: e.tensor_scalar(sb_tmp4[:], ps_den[:], 1e-16, None, ALU.add))
                    op("v", lambda e: e.drain())
                    vvv = op("v", lambda e, buf=buf: e.reciprocal(sb_rden[:, buf, :], sb_tmp4[:]), "vv")
                    pden_free_vv = vvv

                    # --- alpha-scale G rows in place (v: heads 0-1, a: heads 2-3) ---
                    scale_v = {}
                    scale_a = {}
                    for k in range(KMAX):
                        vvv = None
                        aav2 = None
                        for h in range(2):
                            vvv = op("v", lambda e, k=k, h=h, buf=buf: e.tensor_scalar(
                                sb_Gt[:, buf, k, h * 256:(h + 1) * 256],
                                sb_Gt[:, buf, k, h * 256:(h + 1) * 256],
                                sb_alf[:, buf, k, h:h + 1], None, ALU.mult),
                                "vv" if h == 1 else None)
                        for h in range(2, H):
                            aav2 = op("a", lambda e, k=k, h=h, buf=buf: e.activation(
                                sb_Gt[:, buf, k, h * 256:(h + 1) * 256],
                                sb_Gt[:, buf, k, h * 256:(h + 1) * 256],
                                ACT.Copy, scale=sb_alf[:, buf, k, h:h + 1]),
                                "aa" if h == H - 1 else None)
                        scale_v[k] = vvv
                        scale_a[k] = aav2
                    alf_free[buf] = (scale_v[KMAX - 1], scale_a[KMAX - 1])

                    # --- message matmuls (t): 2 x 512 cols, accumulate over k ---
                    ch_mm = None
                    for k in range(KMAX):
                        wt("t", "vv", scale_v[k])
                        wt("t", "aa", scale_a[k])
                        if k == 0:
                            wt("t", "vv", psum_free_vv[0])
                            wt("t", "vv", psum_free_vv[1])
                        for half in range(2):
                            ch_mm = op("t", lambda e, k=k, half=half, buf=buf: e.matmul(
                                pb[half][:],
                                sb_msk[:, buf, k, :],
                                sb_Gt[:, buf, k, half * 512:(half + 1) * 512],
                                start=(k == 0), stop=(k == KMAX - 1)),
                                "mm" if half == 1 else None)
                    msg_mm[t] = ch_mm
                    Gt_free_mm[buf] = ch_mm
                    msk_free_mm[buf] = ch_mm
                    if t + 2 < TPC:
                        rec_gather(t + 2, xh_g=xh_g, xh_sc=xh_sc)

                    # --- epilogue (v) ---
                    wt("v", "mm", msg_mm[t])
                    wt("v", "mm", hnm_free_mm[buf])
                    op("v", lambda e: e.drain())
                    vvv = None
                    for h in range(H):
                        psl = pb[h // 2][:, (h % 2) * 256:(h % 2) * 256 + 256]
                        vvv = op("v", lambda e, h=h, buf=buf, psl=psl: e.tensor_scalar(
                            sb_ep1[:, h * 256:(h + 1) * 256], psl,
                            sb_rden[:, buf, h:h + 1], None, ALU.mult),
                            "vv" if h == H - 1 else None)
                    psum_free_vv[0] = psum_free_vv[1] = vvv
                    rden_free_vv[buf] = vvv
                    op("v", lambda e: e.drain())
                    op("v", lambda e, l=l: e.tensor_add(sb_ep2[:], sb_ep1[:], sb_bias[:, l, :]))
                    op("v", lambda e: e.drain())
                    if l < 2:
                        op("v", lambda e: e.tensor_scalar(sb_ep1[:], sb_ep2[:], 0.0, None, ALU.max))
                        vv2 = op("v", lambda e: e.tensor_scalar(sb_ep3[:], sb_ep2[:], 0.0, None, ALU.min), "vv")
                        wt("a", "vv", vv2)
                        aav = op("a", lambda e: e.activation(sb_ep2[:], sb_ep3[:], ACT.Exp), "aa")
                        wt("v", "aa", aav)
                        op("v", lambda e: e.drain())
                        op("v", lambda e: e.tensor_add(sb_ep3[:], sb_ep1[:], sb_ep2[:]))
                        op("v", lambda e: e.drain())
                        vv2 = op("v", lambda e, buf=buf: e.tensor_scalar(
                            sb_hnm[:, buf, :], sb_ep3[:], -1.0, None, ALU.add), "vv")
                    else:
                        vv2 = op("v", lambda e, buf=buf: e.tensor_copy(sb_hnm[:, buf, :], sb_ep2[:]), "vv")
                    hnm_ready[t] = vv2

                    # --- T-step for this tile (t + v) ---
                    if do_T:
                        wt("t", "vv", hnm_ready[t])
                        last_T = None
                        vvv = None
                        for c in range(HCP):
                            sl = c % 2
                            wt("t", "vv", pbT_free_vv[sl])
                            mmv = op("t", lambda e, c=c, sl=sl, buf=buf: e.matmul(
                                pbT[sl][:], sb_hnm[:, buf, c * 128:(c + 1) * 128],
                                sb_ident[:]), "mm")
                            last_T = mmv
                            wt("v", "mm", mmv)
                            vvv = op("v", lambda e, c=c, sl=sl, t=t: e.tensor_copy(
                                sb_hT[:, c, t * 128:(t + 1) * 128], pbT[sl][:]), "vv")
                            pbT_free_vv[sl] = vvv
                        hnm_free_mm[buf] = last_T
                        hTcol_vv[(l, t)] = vvv

                        # --- interleaved A(l+1) for this bin + AG(l+1) groups ---
                        if l + 1 < NLAYERS:
                            rec_A_bin(l + 1, t, hTcol_vv[(l, t)])
                            if t == TPC - 1 and l + 2 < NLAYERS:
                                rec_lin_reload(l + 2)
                if not do_T:
                    break
            hT_ready_vv = cnt["vv"]

            # ============ OUT-step ============
            grp = 0
            for (f0, f1) in NF:
                slot = grp % 2
                psl = pb[slot][:, 0:f1 - f0]
                wt("t", "vv", psum_free_vv[slot])
                wt("t", "vv", hT_ready_vv)
                mmv = None
                for c in range(HCP):
                    mmv = op("t", lambda e, psl=psl, c=c, f0=f0, f1=f1:
                             e.matmul(psl, sb_wout[:, c, :], sb_hT[:, c, f0:f1],
                                      start=(c == 0), stop=(c == HCP - 1)),
                             "mm" if c == HCP - 1 else None)
                wt("v", "mm", mmv)
                vvv = op("v", lambda e, psl=psl, f0=f0, f1=f1: e.tensor_scalar(
                    sb_osb[:, f0:f1], psl, sb_boutT[:, 0:1], None, ALU.add), "vv")
                psum_free_vv[slot] = vvv
                grp += 1
            wt("s", "vv", cnt["vv"])
            op("s", lambda e: e.dma_start(p_out[:], sb_osb[:]), "xdA", 16)
            wt("s", "xdA", cnt["xdA"])

        # ============ replay ============
        def replay(eng_name):
            def run(e):
                if eng_name == "g":
                    e.load_library(mlp)
                for rec in prog[eng_name]:
                    if rec[0] == "wait":
                        e.wait_ge(sem[rec[1]], rec[2])
                    else:
                        _, fn, inc, amt = rec
                        inst = fn(e)
                        if inc:
                            inst.then_inc(sem[inc], amt)
            return run

        block.gpsimd(replay("g"))
        block.tensor(replay("t"))
        block.vector(replay("v"))
        block.scalar(replay("a"))
        block.sync(replay("s"))

    nc.compile()
    return nc


# =================== host-side data prep ===================

def prep(cfg: Cfg, x, edge_index, node_type, emb_node, w_in, b_in,
         lins, att_ss, att_ds, biases, w_out, b_out):
    """Returns (in_maps, glob) where glob[n] is the packed global row of node n.
    Sets cfg.KMAX. All numpy."""
    N = x.shape[0]
    H, HID, HC, RW, SCW = cfg.H, cfg.HID, cfg.HC, cfg.RW, cfg.SCW
    src = np.concatenate([np.asarray(edge_index[0]), np.arange(N)]).astype(np.int64)
    dst = np.concatenate([np.asarray(edge_index[1]), np.arange(N)]).astype(np.int64)

    deg = np.bincount(dst, minlength=N)
    order = np.argsort(-deg, kind="stable")
    nb = cfg.NBINS
    bin_edges = np.zeros(nb, dtype=np.int64)
    bin_nodes = np.zeros(nb, dtype=np.int64)
    bin_of = np.zeros(N, dtype=np.int64)
    slot_of = np.zeros(N, dtype=np.int64)
    import heapq
    heap = [(0, b) for b in range(nb)]
    heapq.heapify(heap)
    for n in order:
        while True:
            w, b = heapq.heappop(heap)
            if bin_nodes[b] < 128:
                break
        bin_of[n] = b
        slot_of[n] = bin_nodes[b]
        bin_nodes[b] += 1
        bin_edges[b] += deg[n]
        heapq.heappush(heap, (int(bin_edges[b]), b))
    glob = bin_of * 128 + slot_of
    AGS = int(os.environ.get("GAT_AGSPLIT", "5"))
    gsz = (cfg.TPC + AGS - 1) // AGS
    lb = bin_of % cfg.TPC
    grp = lb // gsz
    glob_xh = (grp * gsz * cfg.NCORES * 128 + (bin_of // cfg.TPC) * gsz * 128
               + (lb - grp * gsz) * 128 + slot_of)

    kmax = int(cdiv(int(bin_edges.max()), 128))
    cfg.KMAX = max(kmax, 1)
    KMAX = cfg.KMAX
    TPC, NPC, NSLOT = cfg.TPC, cfg.NPC, cfg.NSLOT

    eb = bin_of[dst]
    eorder = np.argsort(eb, kind="stable")
    es, ed = src[eorder], dst[eorder]
    ebs = eb[eorder]
    starts = np.searchsorted(ebs, np.arange(nb))
    ends = np.searchsorted(ebs, np.arange(nb) + 1)

    CAP = KMAX * 128
    src_g = np.zeros((nb, CAP), dtype=np.int16)
    dst_g = np.zeros((nb, CAP), dtype=np.int16)
    dslot = np.full((nb, CAP), -1, dtype=np.int64)
    for b in range(nb):
        s0, s1 = starts[b], ends[b]
        cntb = s1 - s0
        src_g[b, :cntb] = glob_xh[es[s0:s1]]
        dst_g[b, :cntb] = glob_xh[ed[s0:s1]]
        dslot[b, :cntb] = slot_of[ed[s0:s1]]

    def wrap_idx(flat):
        blk = flat.reshape(TPC * KMAX, 8, 16)
        out = np.zeros((128, TPC * KMAX * 8), dtype=np.int16)
        for gg in range(8):
            out[gg * 16:(gg + 1) * 16, :] = np.transpose(blk, (2, 0, 1)).reshape(16, -1)
        return out

    in_maps = []
    f32 = np.float32
    bf = ml_dtypes.bfloat16

    DIN = cfg.DIN
    DINP = cdiv(DIN, 128)
    HIDP = HID // 128
    HCP = HC // 128
    ACOLS = HC + 8

    X = np.concatenate([np.asarray(x, f32), np.asarray(emb_node, f32)[np.asarray(node_type)]], 1)
    XT = np.zeros((DIN, NSLOT), f32)
    XT[:, glob] = X.T

    lin_augs = []
    for l in range(3):
        lin = np.asarray(lins[l], f32)
        a_sf = np.stack([lin[:, h * HID:(h + 1) * HID] @ np.asarray(att_ss[l], f32)[h] for h in range(H)], 1)
        a_df = np.stack([lin[:, h * HID:(h + 1) * HID] @ np.asarray(att_ds[l], f32)[h] for h in range(H)], 1)
        la = np.concatenate([lin, a_sf, a_df], 1)
        inch_p = HIDP if l == 0 else HCP
        lin_augs.append(la.reshape(inch_p, 128, ACOLS).astype(bf))

    w_in_r = np.asarray(w_in, f32).reshape(DINP, 128, HID).astype(bf)
    b_inT = np.ascontiguousarray(np.asarray(b_in, f32).reshape(HIDP, 128).T)
    bias_bcs = [np.tile(np.asarray(biases[l], f32)[None, :], (128, 1)).astype(f32) for l in range(3)]
    w_out_r = np.asarray(w_out, f32).reshape(HCP, 128, cfg.OUT).astype(bf)
    b_outT = np.asarray(b_out, f32).reshape(cfg.OUT, 1).astype(f32)
    ident = np.eye(128, dtype=bf)
    iota_bc = np.tile(np.arange(128, dtype=bf)[None, :], (128, 1))

    for c in range(cfg.NCORES):
        b0 = c * TPC
        isrc = wrap_idx(src_g[b0:b0 + TPC].reshape(-1))
        idst = wrap_idx(dst_g[b0:b0 + TPC].reshape(-1))
        # dslot column table: [128 partitions(edge slot in chunk), TPC*KMAX]
        ds = dslot[b0:b0 + TPC].reshape(TPC * KMAX, 128)   # [chunk, j]
        dsl = np.ascontiguousarray(ds.T).astype(f32)       # [128, TPC*KMAX]
        in_maps.append({
            "in_augT": XT[:, c * NPC:(c + 1) * NPC].reshape(DINP, 128, NPC).astype(bf),
            "w_in": w_in_r, "b_inT": b_inT,
            "lin0": lin_augs[0], "lin1": lin_augs[1], "lin2": lin_augs[2],
            "bias_bc0": bias_bcs[0], "bias_bc1": bias_bcs[1], "bias_bc2": bias_bcs[2],
            "w_out": w_out_r, "b_outT": b_outT, "ident": ident,
            "idx_src": isrc, "idx_dst": idst,
            "dslot": dsl, "iota_bc": iota_bc,
        })
    return in_maps, glob


def unpack_output(cfg: Cfg, results, glob, N):
    full = np.concatenate([np.asarray(r["outT"]) for r in results], 1)  # [OUT, NSLOT]
    return np.ascontiguousarray(full[:, glob].T.astype(np.float32))


# =================== harness entry point ===================

def kernel(**inputs):
    import numpy as np
    from concourse.bass_utils import run_bass_kernel_spmd

    x = np.asarray(inputs["x"], np.float32)
    N = x.shape[0]
    cfg = Cfg(TPC=10)
    in_maps, glob = prep(
        cfg, x, np.asarray(inputs["edge_index"]), np.asarray(inputs["node_type"]),
        np.asarray(inputs["emb_node"]), np.asarray(inputs["w_in"]), np.asarray(inputs["b_in"]),
        [np.asarray(inputs[f"lin{i}"]) for i in range(3)],
        [np.asarray(inputs[f"att_s{i}"]) for i in range(3)],
        [np.asarray(inputs[f"att_d{i}"]) for i in range(3)],
        [np.asarray(inputs[f"bias{i}"]) for i in range(3)],
        np.asarray(inputs["w_out"]), np.asarray(inputs["b_out"]))
    nc = build_graph(cfg)
    res = run_bass_kernel_spmd(nc, in_maps, core_ids=list(range(cfg.NCORES)))
    return unpack_output(cfg, res.results, glob, N)



# revision 24
# speedup vs baseline: 1.3202x; 1.3202x over previous
"""Distributed 3-layer GAT kernel for TRN2 (8 NeuronCores), v2.

Node layout: nodes greedy-packed into NBINS = NCORES*TPC bins of <=128 slots,
balanced by in-degree. Global row of node n = bin*128 + slot; core c owns bins
[c*TPC,(c+1)*TPC) = rows [c*NPC,(c+1)*NPC).

Per layer:
  A-step : xh[, a_s, a_d] = h @ [lin | att folds]  (TensorE); scores kept as
           raw f32 bytes in bf16 cols [HC, HC+16) via bitcast. DMA to xh_loc,
           grouped AllGather -> xh_full [NSLOT, RW] bf16.
  C-step : per dst tile t (double-buffered): ONE whole-tile dma_gather of src
           rows (RW wide, scores ride along) + ONE dst-score gather (SCW wide).
           Batched score math -> alpha; masks built on-chip via
           is_equal(iota, dslot); denominator via mask lhsT matmuls; alpha
           folded into G rows in-place; 2 message matmuls of 512 cols per
           chunk accumulate in PSUM; epilogue *1/denom, +bias, ELU; T-step
           (transpose to hT) interleaved per tile.
"""
import sys
sys.path.insert(0, "/opt/trn_rl_repo")
import os
from dataclasses import dataclass

import numpy as np
import ml_dtypes

import concourse.bass as bass
import concourse.bacc as bacc
import concourse.mybir as mybir
from concourse.library_config import mlp

BF16 = mybir.dt.bfloat16
F32 = mybir.dt.float32
I16 = mybir.dt.int16
ALU = mybir.AluOpType
ACT = mybir.ActivationFunctionType


@dataclass
class Cfg:
    NCORES: int = 8
    TPC: int = 10
    H: int = 4
    HID: int = 256
    D: int = 384
    OUT: int = 128
    KMAX: int = 17

    @property
    def HC(self):
        return self.H * self.HID

    @property
    def SCW(self):
        return int(__import__('os').environ.get('GAT_SCW', '128'))

    @property
    def RW(self):
        return self.HC + self.SCW

    @property
    def NBINS(self):
        return self.NCORES * self.TPC

    @property
    def NSLOT(self):
        return self.NBINS * 128

    @property
    def NPC(self):
        return self.TPC * 128

    @property
    def DIN(self):
        return self.D + self.HID


def cdiv(a, b):
    return (a + b - 1) // b


def build_graph(cfg: Cfg):
    PHASES = int(os.environ.get("GAT_PHASES", "4"))
    H, HID, HC, RW, SCW = cfg.H, cfg.HID, cfg.HC, cfg.RW, cfg.SCW
    TPC, KMAX, NPC, NSLOT, OUT = cfg.TPC, cfg.KMAX, cfg.NPC, cfg.NSLOT, cfg.OUT
    DINP = cdiv(cfg.DIN, 128)
    HCP = HC // 128
    HIDP = HID // 128
    ACOLS = HC + 8
    assert cfg.DIN % 128 == 0 and HC % 128 == 0 and HID % 128 == 0

    nc = bacc.Bacc("TRN2")

    p_inaug = nc.declare_dram_parameter("in_augT", [DINP, 128, NPC], BF16, isOutput=False)
    p_win = nc.declare_dram_parameter("w_in", [DINP, 128, HID], BF16, isOutput=False)
    p_binT = nc.declare_dram_parameter("b_inT", [128, HIDP], F32, isOutput=False)
    p_lin = [nc.declare_dram_parameter(f"lin{l}", [HIDP if l == 0 else HCP, 128, ACOLS], BF16, isOutput=False) for l in range(3)]
    p_bias = [nc.declare_dram_parameter(f"bias_bc{l}", [128, HC], F32, isOutput=False) for l in range(3)]
    p_wout = nc.declare_dram_parameter("w_out", [HCP, 128, OUT], BF16, isOutput=False)
    p_boutT = nc.declare_dram_parameter("b_outT", [128, 1], F32, isOutput=False)
    p_ident = nc.declare_dram_parameter("ident", [128, 128], BF16, isOutput=False)
    p_isrc = nc.declare_dram_parameter("idx_src", [128, TPC * KMAX * 8], I16, isOutput=False)
    p_idst = nc.declare_dram_parameter("idx_dst", [128, TPC * KMAX * 8], I16, isOutput=False)
    p_dslot = nc.declare_dram_parameter("dslot", [128, TPC * KMAX], F32, isOutput=False)
    p_iota = nc.declare_dram_parameter("iota_bc", [128, 128], BF16, isOutput=False)
    p_out = nc.declare_dram_parameter("outT", [128, NPC], F32, isOutput=True)

    xh_loc = nc.dram_tensor("xh_loc", [NPC, RW], BF16)
    xh_full2 = [nc.dram_tensor(f"xh_full{i}", [NSLOT, RW], BF16, addr_space="Shared")
                for i in range(2)]

    from contextlib import ExitStack
    st = ExitStack()
    with st:
        sb_inaug = st.enter_context(nc.sbuf_tensor("sb_inaug", [128, DINP, NPC], BF16))
        sb_win = st.enter_context(nc.sbuf_tensor("sb_win", [128, DINP, HID], BF16))
        sb_binT = st.enter_context(nc.sbuf_tensor("sb_binT", [128, HIDP], F32))
        sb_lin = st.enter_context(nc.sbuf_tensor("sb_lin", [128, HCP, ACOLS], BF16))
        sb_bias = st.enter_context(nc.sbuf_tensor("sb_bias", [128, 3, HC], F32))
        sb_wout = st.enter_context(nc.sbuf_tensor("sb_wout", [128, HCP, OUT], BF16))
        sb_boutT = st.enter_context(nc.sbuf_tensor("sb_boutT", [128, 1], F32))
        sb_ident = st.enter_context(nc.sbuf_tensor("sb_ident", [128, 128], BF16))
        sb_isrc = st.enter_context(nc.sbuf_tensor("sb_isrc", [128, TPC * KMAX * 8], I16))
        sb_idst = st.enter_context(nc.sbuf_tensor("sb_idst", [128, TPC * KMAX * 8], I16))
        sb_dslot = st.enter_context(nc.sbuf_tensor("sb_dslot", [128, TPC * KMAX], F32))
        sb_iota = st.enter_context(nc.sbuf_tensor("sb_iota", [128, 128], BF16))
        sb_hT = st.enter_context(nc.sbuf_tensor("sb_hT", [128, HCP, NPC], BF16))
        sb_hnm = st.enter_context(nc.sbuf_tensor("sb_hnm", [128, 2, HC], BF16))
        sb_stage = st.enter_context(nc.sbuf_tensor("sb_stage", [128, 2, RW], BF16))
        sb_Gt = st.enter_context(nc.sbuf_tensor("sb_Gt", [128, 2, KMAX, RW], BF16))
        sb_Sd = st.enter_context(nc.sbuf_tensor("sb_Sd", [128, 2, KMAX, SCW], BF16))
        sb_msk = st.enter_context(nc.sbuf_tensor("sb_msk", [128, 2, KMAX, 128], BF16))
        sb_W4 = st.enter_context(nc.sbuf_tensor("sb_W4", [128, 2, 4, 128], BF16))
        sb_alf = st.enter_context(nc.sbuf_tensor("sb_alf", [128, 2, KMAX, 4], F32))
        sb_alb = st.enter_context(nc.sbuf_tensor("sb_alb", [128, 2, KMAX, 4], BF16))
        sb_sc1 = st.enter_context(nc.sbuf_tensor("sb_sc1", [128, KMAX, 4], F32))
        sb_sc2 = st.enter_context(nc.sbuf_tensor("sb_sc2", [128, KMAX, 4], F32))
        sb_sc3 = st.enter_context(nc.sbuf_tensor("sb_sc3", [128, KMAX, 4], F32))
        sb_tmp4 = st.enter_context(nc.sbuf_tensor("sb_tmp4", [128, 4], F32))
        sb_rden = st.enter_context(nc.sbuf_tensor("sb_rden", [128, 2, 4], F32))
        sb_ep1 = st.enter_context(nc.sbuf_tensor("sb_ep1", [128, HC], F32))
        sb_ep2 = st.enter_context(nc.sbuf_tensor("sb_ep2", [128, HC], F32))
        sb_ep3 = st.enter_context(nc.sbuf_tensor("sb_ep3", [128, HC], F32))
        sb_osb = st.enter_context(nc.sbuf_tensor("sb_osb", [128, NPC], F32))
        pb = [st.enter_context(nc.psum_tensor(f"pb{i}", [128, 512], F32)) for i in range(4)]
        pbT = [st.enter_context(nc.psum_tensor(f"pbT{i}", [128, 128], F32)) for i in range(2)]
        ps_a3 = st.enter_context(nc.psum_tensor("ps_a3", [128, 8], F32))
        ps_den = st.enter_context(nc.psum_tensor("ps_den", [128, 4], F32))
        s_pdma = st.enter_context(nc.semaphore("pdma"))
        s_gthA = st.enter_context(nc.semaphore("gthA"))
        s_gthB = st.enter_context(nc.semaphore("gthB"))
        s_gthC = st.enter_context(nc.semaphore("gthC"))
        s_gthD = st.enter_context(nc.semaphore("gthD"))
        s_xdA = st.enter_context(nc.semaphore("xdA"))
        s_xdB = st.enter_context(nc.semaphore("xdB"))
        s_cc = st.enter_context(nc.semaphore("cc"))
        s_mm = st.enter_context(nc.semaphore("mm"))
        s_vv = st.enter_context(nc.semaphore("vv"))
        s_aa = st.enter_context(nc.semaphore("aa"))
        s_gg = st.enter_context(nc.semaphore("gg"))
        block = st.enter_context(nc.Block())
        sem = {"pdma": s_pdma, "gthA": s_gthA, "gthB": s_gthB,
               "gthC": s_gthC, "gthD": s_gthD,
               "xdA": s_xdA, "xdB": s_xdB, "cc": s_cc,
               "mm": s_mm, "vv": s_vv, "aa": s_aa, "gg": s_gg}
        prog = {"g": [], "t": [], "v": [], "a": [], "s": []}
        cnt = {k: 0 for k in sem}
        reg_cache = {}

        def nreg(e, v):
            key = (id(e), v)
            if key not in reg_cache:
                reg_cache[key] = e.to_reg(v)
            return reg_cache[key]

        def op(eng, fn, inc=None, amt=1):
            prog[eng].append(("op", fn, inc, amt))
            if inc:
                cnt[inc] += amt
                return cnt[inc]
            return None

        def wt(eng, sm, val):
            if val and val > 0:
                prog[eng].append(("wait", sm, val))

        # ============ phase 0: loads ============
        loads = [
            (sb_inaug[:], bass.AP(p_inaug, 0, [[NPC, 128], [128 * NPC, DINP], [1, NPC]])),
            (sb_win[:], bass.AP(p_win, 0, [[HID, 128], [128 * HID, DINP], [1, HID]])),
            (sb_binT[:], p_binT[:]),
            (sb_bias[:, 0, :], p_bias[0][:]),
            (sb_bias[:, 1, :], p_bias[1][:]),
            (sb_bias[:, 2, :], p_bias[2][:]),
            (sb_wout[:], bass.AP(p_wout, 0, [[OUT, 128], [128 * OUT, HCP], [1, OUT]])),
            (sb_boutT[:], p_boutT[:]),
            (sb_ident[:], p_ident[:]),
            (sb_isrc[:], p_isrc[:]),
            (sb_idst[:], p_idst[:]),
            (sb_dslot[:], p_dslot[:]),
            (sb_iota[:], p_iota[:]),
            (sb_lin[:, 0:HIDP, :], bass.AP(p_lin[0], 0, [[ACOLS, 128], [128 * ACOLS, HIDP], [1, ACOLS]])),
        ]
        for d, sr in loads:
            op("s", lambda e, d=d, sr=sr: e.dma_start(d, sr), "pdma", 16)
        pdma_loaded = cnt["pdma"]
        if RW > HC + 16:
            op("v", lambda e: e.memset(sb_stage[:, :, HC + 16:RW], 0), "vv")
        if PHASES < 4:
            op("v", lambda e: e.memset(sb_hT[:], 0), "vv")
            op("v", lambda e: e.memset(sb_hnm[:], 0), "vv")
            op("v", lambda e: e.drain())
        for eng in ("g", "t", "v", "a"):
            wt(eng, "pdma", pdma_loaded)

        # persistent cross-step state
        psum_free_vv = {0: 0, 1: 0, 2: 0, 3: 0}   # pb free-after vv
        pbT_free_aa = {0: 0, 1: 0}
        psa3_free = [0]
        pden_free_vv = 0
        stage_free_xdma = {0: None, 1: None}
        Gt_free_mm = {0: 0, 1: 0}
        W4_free_state = {0: 0, 1: 0}
        Sd_free_vv = {0: 0, 1: 0}
        msk_free_mm = {0: 0, 1: 0}
        alf_free = {0: (0, 0), 1: (0, 0)}   # (vv, aa) after scales of that buf
        rden_free_vv = {0: 0, 1: 0}
        hnm_free_mm = {0: 0, 1: 0}
        hT_ready_vv = 0
        hT_ready_gg = 0

        REPS = int(os.environ.get("GAT_REPS", "1"))
        AGS = int(os.environ.get("GAT_AGSPLIT", "1"))
        GSPLIT = int(os.environ.get("GAT_GSPLIT", "8"))
        GREP = int(os.environ.get("GAT_GREP", "1"))
        SREP = int(os.environ.get("GAT_SREP", "1"))
        CCREP = int(os.environ.get("GAT_CCREP", "1"))
        NLAYERS = 3 if PHASES >= 4 else min(PHASES, 1)
        do_C = PHASES >= 2
        do_T = PHASES >= 3
        NIDX = KMAX * 128
        NF = [(i * 512, min((i + 1) * 512, NPC)) for i in range(cdiv(NPC, 512))]
        gsz = (TPC + AGS - 1) // AGS
        NCR = cfg.NCORES * 128

        pdma_lin = {0: pdma_loaded}
        ccv_layer = {}
        hT_copies = {}   # per (l, tile): [(sem, val)] per hT chunk after T-copies

        def rec_OUT_piece(pi, f0, f1, l):
            slot = 2 + pi % 2
            pst = pb[slot]
            psl = pst[:, 0:f1 - f0]
            gates = hT_copies[(l, (f1 - 1) // 128)]
            wt("t", "vv", psum_free_vv[slot])
            mmv = None
            for c in range(HCP):
                wt("t", gates[c][0], gates[c][1])
                mmv = op("t", lambda e, psl=psl, c=c, f0=f0, f1=f1:
                         e.matmul(psl, sb_wout[:, c, :], sb_hT[:, c, f0:f1],
                                  start=(c == 0), stop=(c == HCP - 1)),
                         "mm" if c == HCP - 1 else None)
            wt("v", "mm", mmv)
            vvv = op("v", lambda e, psl=psl, f0=f0, f1=f1: e.tensor_scalar(
                sb_osb[:, f0:f1], psl, sb_boutT[:, 0:1], None, ALU.add), "vv")
            psum_free_vv[slot] = vvv

        def rec_lin_reload(l):
            wt("s", "mm", cnt["mm"])
            op("s", lambda e, l=l: e.dma_start(
                sb_lin[:, 0:HCP, :],
                bass.AP(p_lin[l], 0, [[ACOLS, 128], [128 * ACOLS, HCP], [1, ACOLS]])), "pdma", 16)
            pdma_lin[l] = cnt["pdma"]

        def rec_A_bin(l, b, hT_gate, hT_gate_c=None, defer_copies=False):
            inch_p = HIDP if l == 0 else HCP
            wt("t", "pdma", pdma_lin[l])
            if hT_gate:
                wt("t", hT_gate[0], hT_gate[1])
            fch = [(0, 512, pb[2], 2), (512, 1024, pb[3], 3), (1024, ACOLS, ps_a3, -1)]
            mmv = None
            for fi, (f0, f1, pst, slot) in enumerate(fch):
                if slot < 0:
                    wt("t", "vv", psa3_free[0])
                else:
                    wt("t", "vv", psum_free_vv[slot])
                psl = pst[:, 0:f1 - f0]
                for c in range(inch_p):
                    if fi == 0 and hT_gate_c is not None:
                        wt("t", hT_gate_c[c][0], hT_gate_c[c][1])
                    mmv = op("t", lambda e, psl=psl, c=c, b=b, f0=f0, f1=f1, inch_p=inch_p:
                             e.matmul(psl, sb_hT[:, c, b * 128:(b + 1) * 128],
                                      sb_lin[:, c, f0:f1],
                                      start=(c == 0), stop=(c == inch_p - 1)),
                             "mm" if c == inch_p - 1 else None)
            pl = pdma_lin[l]

            def emit_copies():
                sslot = b % 2
                wt("v", "pdma", pl)
                wt("v", "mm", mmv)
                if stage_free_xdma[sslot]:
                    wt("v", *stage_free_xdma[sslot])
                op("v", lambda e, sslot=sslot: e.tensor_copy(sb_stage[:, sslot, 0:512], pb[2][:]))
                op("v", lambda e, sslot=sslot: e.tensor_copy(sb_stage[:, sslot, 512:1024], pb[3][:]))
                vvv = op("v", lambda e, sslot=sslot: e.tensor_copy(
                    sb_stage[:, sslot, HC:HC + 16].bitcast(F32), ps_a3[:]), "vv")
                psum_free_vv[2] = psum_free_vv[3] = psa3_free[0] = vvv
                wt("s", "vv", vvv)
                xsem = "xdA" if sslot == 0 else "xdB"
                xdv = op("s", lambda e, b=b, sslot=sslot:
                         e.dma_start(xh_loc[b * 128:(b + 1) * 128, :], sb_stage[:, sslot, :]),
                         xsem, 16)
                stage_free_xdma[sslot] = (xsem, xdv)
                if (b + 1) % gsz == 0 or b == TPC - 1:
                    b0g = (b // gsz) * gsz
                    nbg = b - b0g + 1
                    wt("g", xsem, xdv)
                    if b > 0 and stage_free_xdma[1 - sslot]:
                        wt("g", *stage_free_xdma[1 - sslot])
                    xf = xh_full2[l % 2]
                    for _cc in range(CCREP):
                        ccv_layer[l] = op("g", lambda e, b0g=b0g, nbg=nbg, xf=xf: e.collective_compute(
                            "AllGather", ALU.bypass,
                            replica_groups=[list(range(cfg.NCORES))],
                            ins=[xh_loc[b0g * 128:(b0g + nbg) * 128, :]],
                            outs=[xf[b0g * NCR:(b0g + nbg) * NCR, :]]), "cc", 1)

            if defer_copies:
                return emit_copies
            emit_copies()
            return None

        for rep in range(REPS):
            if rep > 0:
                wt("s", "mm", cnt["mm"])
                wt("s", "vv", cnt["vv"])
                op("s", lambda e: e.dma_start(
                    sb_lin[:, 0:HIDP, :],
                    bass.AP(p_lin[0], 0, [[ACOLS, 128], [128 * ACOLS, HIDP], [1, ACOLS]])), "pdma", 16)
                pdma_lin[0] = cnt["pdma"]
                wt("t", "pdma", pdma_lin[0])

            # ============ IN-step: hT[:, 0:HIDP, :] = (w_in.T @ in_aug) + b_in ============
            grp = 0
            for oc in range(HIDP):
                for (f0, f1) in NF:
                    slot = grp % 2
                    psl = pb[slot][:, 0:f1 - f0]
                    wt("t", "vv", psum_free_vv[slot])
                    for c in range(DINP):
                        mmv = op("t", lambda e, psl=psl, c=c, oc=oc, f0=f0, f1=f1:
                                 e.matmul(psl, sb_win[:, c, oc * 128:(oc + 1) * 128],
                                          sb_inaug[:, c, f0:f1],
                                          start=(c == 0), stop=(c == DINP - 1)),
                                 "mm" if c == DINP - 1 else None)
                    wt("v", "mm", mmv)
                    vvv = op("v", lambda e, psl=psl, oc=oc, f0=f0, f1=f1:
                             e.tensor_scalar(sb_hT[:, oc, f0:f1], psl,
                                             sb_binT[:, oc:oc + 1], None, ALU.add), "vv")
                    psum_free_vv[slot] = vvv
                    grp += 1
            hT_ready_vv = cnt["vv"]

            # ============ prologue: A(0) + AG(0) ============
            for b in range(TPC):
                rec_A_bin(0, b, ("vv", hT_ready_vv))
            if NLAYERS > 1:
                rec_lin_reload(1)

            # ============ layers (C with interleaved T, A(l+1), AG(l+1)) ============
            for l in range(NLAYERS):
                if not do_C:
                    break
                wt("g", "cc", ccv_layer[l])
                xf = xh_full2[l % 2]
                xh_g = bass.AP(xf, 0, [[RW, NSLOT], [1, RW]])
                xh_sc = bass.AP(xf, HC, [[RW, NSLOT], [1, SCW]])
                gtv = {}
                sdv = {}
                den_mm = {}
                msg_mm = {}
                hnm_ready = {}

                def rec_gather(t, xh_g=None, xh_sc=None):
                    buf = t % 2
                    icol = (t * KMAX) * 8
                    gsm = "gthA" if buf == 0 else "gthC"
                    ssm = "gthB" if buf == 0 else "gthD"
                    wt("g", "mm", Gt_free_mm[buf])
                    wt("g", gsm, gtv.get(t - 2, 0))
                    for _gr in range(GREP):
                        for k0 in range(0, KMAX, GSPLIT):
                            nch = min(GSPLIT, KMAX - k0)
                            gtv[t] = op("g", lambda e, icol=icol, buf=buf, k0=k0, nch=nch, xh_g=xh_g: e.dma_gather(
                                sb_Gt[:, buf, k0:k0 + nch, :], xh_g,
                                sb_isrc[:, icol + k0 * 8:icol + (k0 + nch) * 8],
                                nch * 128, nreg(e, nch * 128), RW, elem_step=RW), gsm, 16)
                    wt("g", "vv", Sd_free_vv[buf])
                    wt("g", ssm, sdv.get(t - 2, 0))
                    for _sr in range(SREP):
                        for k0 in range(0, KMAX, GSPLIT):
                            nch = min(GSPLIT, KMAX - k0)
                            sdv[t] = op("g", lambda e, icol=icol, buf=buf, k0=k0, nch=nch, xh_sc=xh_sc: e.dma_gather(
                                sb_Sd[:, buf, k0:k0 + nch, :], xh_sc,
                                sb_idst[:, icol + k0 * 8:icol + (k0 + nch) * 8],
                                nch * 128, nreg(e, nch * 128), SCW, elem_step=RW), ssm, 16)

                rec_gather(0, xh_g=xh_g, xh_sc=xh_sc)
                rec_gather(1, xh_g=xh_g, xh_sc=xh_sc)

                def rec_tail(tt, l=l):
                    """Epilogue + T + A(l+1)/OUT for tile tt; returns deferred
                    A stage-copy closure (or None)."""
                    buf = tt % 2
                    # --- epilogue (v + a) ---
                    wt("v", "mm", msg_mm[tt])
                    wt("v", "mm", hnm_free_mm[buf])
                    op("v", lambda e: e.drain())
                    vvv = None
                    for h in range(H):
                        psl = pb[h // 2][:, (h % 2) * 256:(h % 2) * 256 + 256]
                        vvv = op("v", lambda e, h=h, buf=buf, psl=psl: e.tensor_scalar(
                            sb_ep1[:, h * 256:(h + 1) * 256], psl,
                            sb_rden[:, buf, h:h + 1], None, ALU.mult),
                            "vv" if h == H - 1 else None)
                    psum_free_vv[0] = psum_free_vv[1] = vvv
                    rden_free_vv[buf] = vvv
                    op("v", lambda e: e.drain())
                    op("v", lambda e, l=l: e.tensor_add(sb_ep2[:], sb_ep1[:], sb_bias[:, l, :]))
                    op("v", lambda e: e.drain())
                    if l < 2:
                        op("v", lambda e: e.tensor_scalar(sb_ep1[:], sb_ep2[:], 0.0, None, ALU.max))
                        vv2 = op("v", lambda e: e.tensor_scalar(sb_ep3[:], sb_ep2[:], 0.0, None, ALU.min), "vv")
                        wt("a", "vv", vv2)
                        aav = op("a", lambda e: e.activation(sb_ep2[:], sb_ep3[:], ACT.Exp), "aa")
                        wt("v", "aa", aav)
                        op("v", lambda e: e.drain())
                        op("v", lambda e: e.tensor_add(sb_ep3[:], sb_ep1[:], sb_ep2[:]))
                        op("v", lambda e: e.drain())
                        vv2 = op("v", lambda e, buf=buf: e.tensor_scalar(
                            sb_hnm[:, buf, :], sb_ep3[:], -1.0, None, ALU.add), "vv")
                    else:
                        vv2 = op("v", lambda e, buf=buf: e.tensor_copy(sb_hnm[:, buf, :], sb_ep2[:]), "vv")
                    hnm_ready[tt] = vv2

                    # --- T-step (t + a copies) ---
                    wt("t", "vv", hnm_ready[tt])
                    last_T = None
                    copies = []
                    for c in range(HCP):
                        sl = c % 2
                        wt("t", "aa", pbT_free_aa[sl])
                        mmv = op("t", lambda e, c=c, sl=sl, buf=buf: e.matmul(
                            pbT[sl][:], sb_hnm[:, buf, c * 128:(c + 1) * 128],
                            sb_ident[:]), "mm")
                        last_T = mmv
                        wt("a", "mm", mmv)
                        aav3 = op("a", lambda e, c=c, sl=sl, tt=tt: e.activation(
                            sb_hT[:, c, tt * 128:(tt + 1) * 128], pbT[sl][:],
                            ACT.Copy), "aa")
                        pbT_free_aa[sl] = aav3
                        copies.append(("aa", aav3))
                    hnm_free_mm[buf] = last_T
                    hT_copies[(l, tt)] = copies

                    deferred = None
                    if l + 1 < NLAYERS:
                        deferred = rec_A_bin(l + 1, tt, None, hT_gate_c=copies,
                                             defer_copies=True)
                        if tt == TPC - 1 and l + 2 < NLAYERS:
                            rec_lin_reload(l + 2)
                    elif l == NLAYERS - 1 and NLAYERS == 3:
                        for pi, (f0, f1) in enumerate(NF):
                            if tt == (f1 - 1) // 128:
                                rec_OUT_piece(pi, f0, f1, l)
                    return deferred

                for t in range(TPC):
                    buf = t % 2
                    # --- masks for this tile (v) ---
                    wt("v", "mm", msk_free_mm[buf])
                    mskv = None
                    for k in range(KMAX):
                        mskv = op("v", lambda e, t=t, k=k, buf=buf: e.tensor_scalar(
                            sb_msk[:, buf, k, :], sb_iota[:],
                            sb_dslot[:, t * KMAX + k:t * KMAX + k + 1], None, ALU.is_equal),
                            "vv" if k == KMAX - 1 else None)
                    msk_ready = mskv
                    # --- score math (v + a) ---
                    wt("v", "gthA" if buf == 0 else "gthC", gtv[t])
                    wt("v", "gthB" if buf == 0 else "gthD", sdv[t])
                    av, aa_ = alf_free[buf]
                    wt("v", "vv", av)
                    wt("v", "aa", aa_)
                    GtF = sb_Gt[:, buf, :, :].bitcast(F32)   # [128, KMAX, RW//2]
                    SdF = sb_Sd[:, buf, :, :].bitcast(F32)   # [128, KMAX, SCW//2]
                    op("v", lambda e, GtF=GtF, SdF=SdF: e.tensor_add(
                        sb_sc1[:], GtF[:, :, HC // 2:HC // 2 + 4], SdF[:, :, 4:8]))
                    op("v", lambda e: e.drain())
                    op("v", lambda e: e.tensor_scalar(sb_sc2[:], sb_sc1[:], 0.0, None, ALU.max))
                    op("v", lambda e: e.tensor_scalar(sb_sc3[:], sb_sc1[:], 0.0, 0.2, ALU.min, ALU.mult))
                    op("v", lambda e: e.drain())
                    vvv = op("v", lambda e: e.tensor_add(sb_sc1[:], sb_sc2[:], sb_sc3[:]), "vv")
                    wt("a", "vv", vvv)
                    aav = op("a", lambda e, buf=buf: e.activation(
                        sb_alf[:, buf, :, :], sb_sc1[:], ACT.Exp), "aa")
                    wt("v", "aa", aav)
                    vvv = op("v", lambda e, buf=buf: e.tensor_copy(
                        sb_alb[:, buf, :, :], sb_alf[:, buf, :, :]), "vv")
                    alb_ready = vvv
                    Sd_free_vv[buf] = vvv

                    # --- denominator (t) ---
                    wt("t", "vv", alb_ready)
                    wt("t", "vv", msk_ready)
                    wt("t", "vv", pden_free_vv)
                    mmv = None
                    for k in range(KMAX):
                        mmv = op("t", lambda e, k=k, buf=buf: e.matmul(
                            ps_den[:], sb_msk[:, buf, k, :], sb_alb[:, buf, k, :],
                            start=(k == 0), stop=(k == KMAX - 1)),
                            "mm" if k == KMAX - 1 else None)
                    den_mm[t] = mmv

                    # --- deferred tail of previous tile (epi + T + A/OUT) ---
                    if do_T and t > 0:
                        pend_copies = rec_tail(t - 1)
                    else:
                        pend_copies = None

                    # --- alpha-scale G rows in place (v: heads 0-1, a: heads 2-3) ---
                    scale_v = {}
                    scale_a = {}
                    for k in range(KMAX):
                        vvv = None
                        aav2 = None
                        for h in range(2):
                            vvv = op("v", lambda e, k=k, h=h, buf=buf: e.tensor_scalar(
                                sb_Gt[:, buf, k, h * 256:(h + 1) * 256],
                                sb_Gt[:, buf, k, h * 256:(h + 1) * 256],
                                sb_alf[:, buf, k, h:h + 1], None, ALU.mult),
                                "vv" if h == 1 else None)
                        for h in range(2, H):
                            aav2 = op("a", lambda e, k=k, h=h, buf=buf: e.activation(
                                sb_Gt[:, buf, k, h * 256:(h + 1) * 256],
                                sb_Gt[:, buf, k, h * 256:(h + 1) * 256],
                                ACT.Copy, scale=sb_alf[:, buf, k, h:h + 1]),
                                "aa" if h == H - 1 else None)
                        scale_v[k] = vvv
                        scale_a[k] = aav2
                    alf_free[buf] = (scale_v[KMAX - 1], scale_a[KMAX - 1])

                    # --- reciprocal (v), after scales so v doesn't stall on PE ---
                    wt("v", "mm", den_mm[t])
                    wt("v", "vv", rden_free_vv[buf])
                    op("v", lambda e: e.tensor_scalar(sb_tmp4[:], ps_den[:], 1e-16, None, ALU.add))
                    op("v", lambda e: e.drain())
                    vvv = op("v", lambda e, buf=buf: e.reciprocal(sb_rden[:, buf, :], sb_tmp4[:]), "vv")
                    pden_free_vv = vvv

                    # --- deferred A stage-copies of previous tile (v tail) ---
                    if pend_copies is not None:
                        pend_copies()
                        pend_copies = None

                    # --- message matmuls (t): 2 x 512 cols, accumulate over k ---
                    ch_mm = None
                    for k in range(KMAX):
                        wt("t", "vv", scale_v[k])
                        wt("t", "aa", scale_a[k])
                        if k == 0:
                            wt("t", "vv", psum_free_vv[0])
                            wt("t", "vv", psum_free_vv[1])
                        for half in range(2):
                            ch_mm = op("t", lambda e, k=k, half=half, buf=buf: e.matmul(
                                pb[half][:],
                                sb_msk[:, buf, k, :],
                                sb_Gt[:, buf, k, half * 512:(half + 1) * 512],
                                start=(k == 0), stop=(k == KMAX - 1)),
                                "mm" if half == 1 else None)
                    msg_mm[t] = ch_mm
                    Gt_free_mm[buf] = ch_mm
                    msk_free_mm[buf] = ch_mm
                    if t + 2 < TPC:
                        rec_gather(t + 2, xh_g=xh_g, xh_sc=xh_sc)

                    if not do_T:
                        # --- inline epilogue (ablation path) ---
                        wt("v", "mm", msg_mm[t])
                        op("v", lambda e: e.drain())
                        vvv = None
                        for h in range(H):
                            psl = pb[h // 2][:, (h % 2) * 256:(h % 2) * 256 + 256]
                            vvv = op("v", lambda e, h=h, buf=buf, psl=psl: e.tensor_scalar(
                                sb_ep1[:, h * 256:(h + 1) * 256], psl,
                                sb_rden[:, buf, h:h + 1], None, ALU.mult),
                                "vv" if h == H - 1 else None)
                        psum_free_vv[0] = psum_free_vv[1] = vvv
                        rden_free_vv[buf] = vvv

                # --- flush last tile's tail ---
                if do_T:
                    pend = rec_tail(TPC - 1)
                    if pend is not None:
                        pend()
                if not do_T:
                    break
            hT_ready_vv = cnt["vv"]
            hT_ready_aa = cnt["aa"]

            # ============ OUT-step (fallback when not inlined per tile) ============
            if not (NLAYERS == 3 and do_T):
                grp = 0
                for (f0, f1) in NF:
                    slot = grp % 2
                    psl = pb[slot][:, 0:f1 - f0]
                    wt("t", "vv", psum_free_vv[slot])
                    wt("t", "vv", hT_ready_vv)
                    wt("t", "aa", hT_ready_aa)
                    mmv = None
                    for c in range(HCP):
                        mmv = op("t", lambda e, psl=psl, c=c, f0=f0, f1=f1:
                                 e.matmul(psl, sb_wout[:, c, :], sb_hT[:, c, f0:f1],
                                          start=(c == 0), stop=(c == HCP - 1)),
                                 "mm" if c == HCP - 1 else None)
                    wt("v", "mm", mmv)
                    vvv = op("v", lambda e, psl=psl, f0=f0, f1=f1: e.tensor_scalar(
                        sb_osb[:, f0:f1], psl, sb_boutT[:, 0:1], None, ALU.add), "vv")
                    psum_free_vv[slot] = vvv
                    grp += 1
            wt("s", "vv", cnt["vv"])
            op("s", lambda e: e.dma_start(p_out[:], sb_osb[:]), "xdA", 16)
            wt("s", "xdA", cnt["xdA"])

        # ============ replay ============
        def replay(eng_name):
            def run(e):
                if eng_name == "g":
                    e.load_library(mlp)
                for rec in prog[eng_name]:
                    if rec[0] == "wait":
                        e.wait_ge(sem[rec[1]], rec[2])
                    else:
                        _, fn, inc, amt = rec
                        inst = fn(e)
                        if inc:
                            inst.then_inc(sem[inc], amt)
            return run

        block.gpsimd(replay("g"))
        block.tensor(replay("t"))
        block.vector(replay("v"))
        block.scalar(replay("a"))
        block.sync(replay("s"))

    nc.compile()
    return nc


# =================== host-side data prep ===================

def prep(cfg: Cfg, x, edge_index, node_type, emb_node, w_in, b_in,
         lins, att_ss, att_ds, biases, w_out, b_out):
    """Returns (in_maps, glob) where glob[n] is the packed global row of node n.
    Sets cfg.KMAX. All numpy."""
    N = x.shape[0]
    H, HID, HC, RW, SCW = cfg.H, cfg.HID, cfg.HC, cfg.RW, cfg.SCW
    src = np.concatenate([np.asarray(edge_index[0]), np.arange(N)]).astype(np.int64)
    dst = np.concatenate([np.asarray(edge_index[1]), np.arange(N)]).astype(np.int64)

    deg = np.bincount(dst, minlength=N)
    order = np.argsort(-deg, kind="stable")
    nb = cfg.NBINS
    bin_edges = np.zeros(nb, dtype=np.int64)
    bin_nodes = np.zeros(nb, dtype=np.int64)
    bin_of = np.zeros(N, dtype=np.int64)
    slot_of = np.zeros(N, dtype=np.int64)
    import heapq
    heap = [(0, b) for b in range(nb)]
    heapq.heapify(heap)
    for n in order:
        while True:
            w, b = heapq.heappop(heap)
            if bin_nodes[b] < 128:
                break
        bin_of[n] = b
        slot_of[n] = bin_nodes[b]
        bin_nodes[b] += 1
        bin_edges[b] += deg[n]
        heapq.heappush(heap, (int(bin_edges[b]), b))
    glob = bin_of * 128 + slot_of
    AGS = int(os.environ.get("GAT_AGSPLIT", "1"))
    gsz = (cfg.TPC + AGS - 1) // AGS
    lb = bin_of % cfg.TPC
    grp = lb // gsz
    glob_xh = (grp * gsz * cfg.NCORES * 128 + (bin_of // cfg.TPC) * gsz * 128
               + (lb - grp * gsz) * 128 + slot_of)

    kmax = int(cdiv(int(bin_edges.max()), 128))
    cfg.KMAX = max(kmax, 1)
    KMAX = cfg.KMAX
    TPC, NPC, NSLOT = cfg.TPC, cfg.NPC, cfg.NSLOT

    eb = bin_of[dst]
    eorder = np.argsort(eb, kind="stable")
    es, ed = src[eorder], dst[eorder]
    ebs = eb[eorder]
    starts = np.searchsorted(ebs, np.arange(nb))
    ends = np.searchsorted(ebs, np.arange(nb) + 1)

    CAP = KMAX * 128
    src_g = np.zeros((nb, CAP), dtype=np.int16)
    dst_g = np.zeros((nb, CAP), dtype=np.int16)
    dslot = np.full((nb, CAP), -1, dtype=np.int64)
    for b in range(nb):
        s0, s1 = starts[b], ends[b]
        cntb = s1 - s0
        src_g[b, :cntb] = glob_xh[es[s0:s1]]
        dst_g[b, :cntb] = glob_xh[ed[s0:s1]]
        dslot[b, :cntb] = slot_of[ed[s0:s1]]

    def wrap_idx(flat):
        blk = flat.reshape(TPC * KMAX, 8, 16)
        out = np.zeros((128, TPC * KMAX * 8), dtype=np.int16)
        for gg in range(8):
            out[gg * 16:(gg + 1) * 16, :] = np.transpose(blk, (2, 0, 1)).reshape(16, -1)
        return out

    in_maps = []
    f32 = np.float32
    bf = ml_dtypes.bfloat16

    DIN = cfg.DIN
    DINP = cdiv(DIN, 128)
    HIDP = HID // 128
    HCP = HC // 128
    ACOLS = HC + 8

    X = np.concatenate([np.asarray(x, f32), np.asarray(emb_node, f32)[np.asarray(node_type)]], 1)
    XT = np.zeros((DIN, NSLOT), f32)
    XT[:, glob] = X.T

    lin_augs = []
    for l in range(3):
        lin = np.asarray(lins[l], f32)
        a_sf = np.stack([lin[:, h * HID:(h + 1) * HID] @ np.asarray(att_ss[l], f32)[h] for h in range(H)], 1)
        a_df = np.stack([lin[:, h * HID:(h + 1) * HID] @ np.asarray(att_ds[l], f32)[h] for h in range(H)], 1)
        la = np.concatenate([lin, a_sf, a_df], 1)
        inch_p = HIDP if l == 0 else HCP
        lin_augs.append(la.reshape(inch_p, 128, ACOLS).astype(bf))

    w_in_r = np.asarray(w_in, f32).reshape(DINP, 128, HID).astype(bf)
    b_inT = np.ascontiguousarray(np.asarray(b_in, f32).reshape(HIDP, 128).T)
    bias_bcs = [np.tile(np.asarray(biases[l], f32)[None, :], (128, 1)).astype(f32) for l in range(3)]
    w_out_r = np.asarray(w_out, f32).reshape(HCP, 128, cfg.OUT).astype(bf)
    b_outT = np.asarray(b_out, f32).reshape(cfg.OUT, 1).astype(f32)
    ident = np.eye(128, dtype=bf)
    iota_bc = np.tile(np.arange(128, dtype=bf)[None, :], (128, 1))

    for c in range(cfg.NCORES):
        b0 = c * TPC
        isrc = wrap_idx(src_g[b0:b0 + TPC].reshape(-1))
        idst = wrap_idx(dst_g[b0:b0 + TPC].reshape(-1))
        # dslot column table: [128 partitions(edge slot in chunk), TPC*KMAX]
        ds = dslot[b0:b0 + TPC].reshape(TPC * KMAX, 128)   # [chunk, j]
        dsl = np.ascontiguousarray(ds.T).astype(f32)       # [128, TPC*KMAX]
        in_maps.append({
            "in_augT": XT[:, c * NPC:(c + 1) * NPC].reshape(DINP, 128, NPC).astype(bf),
            "w_in": w_in_r, "b_inT": b_inT,
            "lin0": lin_augs[0], "lin1": lin_augs[1], "lin2": lin_augs[2],
            "bias_bc0": bias_bcs[0], "bias_bc1": bias_bcs[1], "bias_bc2": bias_bcs[2],
            "w_out": w_out_r, "b_outT": b_outT, "ident": ident,
            "idx_src": isrc, "idx_dst": idst,
            "dslot": dsl, "iota_bc": iota_bc,
        })
    return in_maps, glob


def unpack_output(cfg: Cfg, results, glob, N):
    full = np.concatenate([np.asarray(r["outT"]) for r in results], 1)  # [OUT, NSLOT]
    return np.ascontiguousarray(full[:, glob].T.astype(np.float32))


# =================== harness entry point ===================

def kernel(**inputs):
    import numpy as np
    from concourse.bass_utils import run_bass_kernel_spmd

    x = np.asarray(inputs["x"], np.float32)
    N = x.shape[0]
    cfg = Cfg(TPC=10)
    in_maps, glob = prep(
        cfg, x, np.asarray(inputs["edge_index"]), np.asarray(inputs["node_type"]),
        np.asarray(inputs["emb_node"]), np.asarray(inputs["w_in"]), np.asarray(inputs["b_in"]),
        [np.asarray(inputs[f"lin{i}"]) for i in range(3)],
        [np.asarray(inputs[f"att_s{i}"]) for i in range(3)],
        [np.asarray(inputs[f"att_d{i}"]) for i in range(3)],
        [np.asarray(inputs[f"bias{i}"]) for i in range(3)],
        np.asarray(inputs["w_out"]), np.asarray(inputs["b_out"]))
    nc = build_graph(cfg)
    res = run_bass_kernel_spmd(nc, in_maps, core_ids=list(range(cfg.NCORES)))
    return unpack_output(cfg, res.results, glob, N)



# revision 30
# speedup vs baseline: 1.3221x; 1.0014x over previous
"""Distributed 3-layer GAT kernel for TRN2 (8 NeuronCores), v2.

Node layout: nodes greedy-packed into NBINS = NCORES*TPC bins of <=128 slots,
balanced by in-degree. Global row of node n = bin*128 + slot; core c owns bins
[c*TPC,(c+1)*TPC) = rows [c*NPC,(c+1)*NPC).

Per layer:
  A-step : xh[, a_s, a_d] = h @ [lin | att folds]  (TensorE); scores kept as
           raw f32 bytes in bf16 cols [HC, HC+16) via bitcast. DMA to xh_loc,
           grouped AllGather -> xh_full [NSLOT, RW] bf16.
  C-step : per dst tile t (double-buffered): ONE whole-tile dma_gather of src
           rows (RW wide, scores ride along) + ONE dst-score gather (SCW wide).
           Batched score math -> alpha; masks built on-chip via
           is_equal(iota, dslot); denominator via mask lhsT matmuls; alpha
           folded into G rows in-place; 2 message matmuls of 512 cols per
           chunk accumulate in PSUM; epilogue *1/denom, +bias, ELU; T-step
           (transpose to hT) interleaved per tile.
"""
import sys
sys.path.insert(0, "/opt/trn_rl_repo")
import os
from dataclasses import dataclass

import numpy as np
import ml_dtypes

import concourse.bass as bass
import concourse.bacc as bacc
import concourse.mybir as mybir
from concourse.library_config import mlp

BF16 = mybir.dt.bfloat16
F32 = mybir.dt.float32
I16 = mybir.dt.int16
ALU = mybir.AluOpType
ACT = mybir.ActivationFunctionType


@dataclass
class Cfg:
    NCORES: int = 8
    TPC: int = 10
    H: int = 4
    HID: int = 256
    D: int = 384
    OUT: int = 128
    KMAX: int = 17

    @property
    def HC(self):
        return self.H * self.HID

    @property
    def SCW(self):
        return int(__import__('os').environ.get('GAT_SCW', '128'))

    @property
    def RW(self):
        return self.HC + self.SCW

    @property
    def NBINS(self):
        return self.NCORES * self.TPC

    @property
    def NSLOT(self):
        return self.NBINS * 128

    @property
    def NPC(self):
        return self.TPC * 128

    @property
    def DIN(self):
        return self.D + self.HID


def cdiv(a, b):
    return (a + b - 1) // b


def build_graph(cfg: Cfg):
    PHASES = int(os.environ.get("GAT_PHASES", "4"))
    H, HID, HC, RW, SCW = cfg.H, cfg.HID, cfg.HC, cfg.RW, cfg.SCW
    TPC, KMAX, NPC, NSLOT, OUT = cfg.TPC, cfg.KMAX, cfg.NPC, cfg.NSLOT, cfg.OUT
    DINP = cdiv(cfg.DIN, 128)
    HCP = HC // 128
    HIDP = HID // 128
    ACOLS = HC + 8
    assert cfg.DIN % 128 == 0 and HC % 128 == 0 and HID % 128 == 0

    nc = bacc.Bacc("TRN2")

    p_inaug = nc.declare_dram_parameter("in_augT", [DINP, 128, NPC], BF16, isOutput=False)
    p_win = nc.declare_dram_parameter("w_in", [DINP, 128, HID], BF16, isOutput=False)
    p_binT = nc.declare_dram_parameter("b_inT", [128, HIDP], F32, isOutput=False)
    p_lin = [nc.declare_dram_parameter(f"lin{l}", [HIDP if l == 0 else HCP, 128, ACOLS], BF16, isOutput=False) for l in range(3)]
    p_bias = [nc.declare_dram_parameter(f"bias_bc{l}", [128, HC], F32, isOutput=False) for l in range(3)]
    p_wout = nc.declare_dram_parameter("w_out", [HCP, 128, OUT], BF16, isOutput=False)
    p_boutT = nc.declare_dram_parameter("b_outT", [128, 1], F32, isOutput=False)
    p_ident = nc.declare_dram_parameter("ident", [128, 128], BF16, isOutput=False)
    p_isrc = nc.declare_dram_parameter("idx_src", [128, TPC * KMAX * 8], I16, isOutput=False)
    p_idst = nc.declare_dram_parameter("idx_dst", [128, TPC * KMAX * 8], I16, isOutput=False)
    p_dslot = nc.declare_dram_parameter("dslot", [128, TPC * KMAX], F32, isOutput=False)
    p_iota = nc.declare_dram_parameter("iota_bc", [128, 128], BF16, isOutput=False)
    p_out = nc.declare_dram_parameter("outT", [128, NPC], F32, isOutput=True)

    xh_loc = nc.dram_tensor("xh_loc", [NPC, RW], BF16)
    xh_full2 = [nc.dram_tensor(f"xh_full{i}", [NSLOT, RW], BF16, addr_space="Shared")
                for i in range(2)]

    from contextlib import ExitStack
    st = ExitStack()
    with st:
        sb_inaug = st.enter_context(nc.sbuf_tensor("sb_inaug", [128, DINP, NPC], BF16))
        sb_win = st.enter_context(nc.sbuf_tensor("sb_win", [128, DINP, HID], BF16))
        sb_binT = st.enter_context(nc.sbuf_tensor("sb_binT", [128, HIDP], F32))
        sb_lin = st.enter_context(nc.sbuf_tensor("sb_lin", [128, HCP, ACOLS], BF16))
        sb_bias = st.enter_context(nc.sbuf_tensor("sb_bias", [128, 3, HC], F32))
        sb_wout = st.enter_context(nc.sbuf_tensor("sb_wout", [128, HCP, OUT], BF16))
        sb_boutT = st.enter_context(nc.sbuf_tensor("sb_boutT", [128, 1], F32))
        sb_ident = st.enter_context(nc.sbuf_tensor("sb_ident", [128, 128], BF16))
        sb_isrc = st.enter_context(nc.sbuf_tensor("sb_isrc", [128, TPC * KMAX * 8], I16))
        sb_idst = st.enter_context(nc.sbuf_tensor("sb_idst", [128, TPC * KMAX * 8], I16))
        sb_dslot = st.enter_context(nc.sbuf_tensor("sb_dslot", [128, TPC * KMAX], F32))
        sb_iota = st.enter_context(nc.sbuf_tensor("sb_iota", [128, 128], BF16))
        sb_hT = st.enter_context(nc.sbuf_tensor("sb_hT", [128, HCP, NPC], BF16))
        sb_hnm = st.enter_context(nc.sbuf_tensor("sb_hnm", [128, 2, HC], BF16))
        sb_stage = st.enter_context(nc.sbuf_tensor("sb_stage", [128, 2, RW], BF16))
        sb_Gt = st.enter_context(nc.sbuf_tensor("sb_Gt", [128, 2, KMAX, RW], BF16))
        sb_Sd = st.enter_context(nc.sbuf_tensor("sb_Sd", [128, 2, KMAX, SCW], BF16))
        sb_msk = st.enter_context(nc.sbuf_tensor("sb_msk", [128, 2, KMAX, 128], BF16))
        sb_W4 = st.enter_context(nc.sbuf_tensor("sb_W4", [128, 2, 4, 128], BF16))
        sb_alf = st.enter_context(nc.sbuf_tensor("sb_alf", [128, 2, KMAX, 4], F32))
        sb_alb = st.enter_context(nc.sbuf_tensor("sb_alb", [128, 2, KMAX, 4], BF16))
        sb_sc1 = st.enter_context(nc.sbuf_tensor("sb_sc1", [128, KMAX, 4], F32))
        sb_sc2 = st.enter_context(nc.sbuf_tensor("sb_sc2", [128, KMAX, 4], F32))
        sb_sc3 = st.enter_context(nc.sbuf_tensor("sb_sc3", [128, KMAX, 4], F32))
        sb_tmp4 = st.enter_context(nc.sbuf_tensor("sb_tmp4", [128, 4], F32))
        sb_rden = st.enter_context(nc.sbuf_tensor("sb_rden", [128, 2, 4], F32))
        sb_ep1 = st.enter_context(nc.sbuf_tensor("sb_ep1", [128, HC], F32))
        sb_ep2 = st.enter_context(nc.sbuf_tensor("sb_ep2", [128, HC], F32))
        sb_ep3 = st.enter_context(nc.sbuf_tensor("sb_ep3", [128, HC], F32))
        sb_osb = st.enter_context(nc.sbuf_tensor("sb_osb", [128, NPC], F32))
        pb = [st.enter_context(nc.psum_tensor(f"pb{i}", [128, 512], F32)) for i in range(4)]
        pbT = [st.enter_context(nc.psum_tensor(f"pbT{i}", [128, 128], F32)) for i in range(2)]
        ps_a3 = st.enter_context(nc.psum_tensor("ps_a3", [128, 8], F32))
        ps_den = st.enter_context(nc.psum_tensor("ps_den", [128, 4], F32))
        s_pdma = st.enter_context(nc.semaphore("pdma"))
        s_gthA = st.enter_context(nc.semaphore("gthA"))
        s_gthB = st.enter_context(nc.semaphore("gthB"))
        s_gthC = st.enter_context(nc.semaphore("gthC"))
        s_gthD = st.enter_context(nc.semaphore("gthD"))
        s_xdA = st.enter_context(nc.semaphore("xdA"))
        s_xdB = st.enter_context(nc.semaphore("xdB"))
        s_cc = st.enter_context(nc.semaphore("cc"))
        s_mm = st.enter_context(nc.semaphore("mm"))
        s_vv = st.enter_context(nc.semaphore("vv"))
        s_aa = st.enter_context(nc.semaphore("aa"))
        s_gg = st.enter_context(nc.semaphore("gg"))
        block = st.enter_context(nc.Block())
        sem = {"pdma": s_pdma, "gthA": s_gthA, "gthB": s_gthB,
               "gthC": s_gthC, "gthD": s_gthD,
               "xdA": s_xdA, "xdB": s_xdB, "cc": s_cc,
               "mm": s_mm, "vv": s_vv, "aa": s_aa, "gg": s_gg}
        prog = {"g": [], "t": [], "v": [], "a": [], "s": []}
        cnt = {k: 0 for k in sem}
        reg_cache = {}

        def nreg(e, v):
            key = (id(e), v)
            if key not in reg_cache:
                reg_cache[key] = e.to_reg(v)
            return reg_cache[key]

        def op(eng, fn, inc=None, amt=1):
            prog[eng].append(("op", fn, inc, amt))
            if inc:
                cnt[inc] += amt
                return cnt[inc]
            return None

        def wt(eng, sm, val):
            if val and val > 0:
                prog[eng].append(("wait", sm, val))

        # ============ phase 0: loads ============
        loads = [
            (sb_inaug[:], bass.AP(p_inaug, 0, [[NPC, 128], [128 * NPC, DINP], [1, NPC]])),
            (sb_win[:], bass.AP(p_win, 0, [[HID, 128], [128 * HID, DINP], [1, HID]])),
            (sb_binT[:], p_binT[:]),
            (sb_bias[:, 0, :], p_bias[0][:]),
            (sb_bias[:, 1, :], p_bias[1][:]),
            (sb_bias[:, 2, :], p_bias[2][:]),
            (sb_wout[:], bass.AP(p_wout, 0, [[OUT, 128], [128 * OUT, HCP], [1, OUT]])),
            (sb_boutT[:], p_boutT[:]),
            (sb_ident[:], p_ident[:]),
            (sb_isrc[:], p_isrc[:]),
            (sb_idst[:], p_idst[:]),
            (sb_dslot[:], p_dslot[:]),
            (sb_iota[:], p_iota[:]),
            (sb_lin[:, 0:HIDP, :], bass.AP(p_lin[0], 0, [[ACOLS, 128], [128 * ACOLS, HIDP], [1, ACOLS]])),
        ]
        for d, sr in loads:
            op("s", lambda e, d=d, sr=sr: e.dma_start(d, sr), "pdma", 16)
        pdma_loaded = cnt["pdma"]
        if RW > HC + 16:
            op("v", lambda e: e.memset(sb_stage[:, :, HC + 16:RW], 0), "vv")
        if PHASES < 4:
            op("v", lambda e: e.memset(sb_hT[:], 0), "vv")
            op("v", lambda e: e.memset(sb_hnm[:], 0), "vv")
            op("v", lambda e: e.drain())
        for eng in ("g", "t", "v", "a"):
            wt(eng, "pdma", pdma_loaded)

        # persistent cross-step state
        psum_free_vv = {0: 0, 1: 0, 2: 0, 3: 0}   # pb free-after vv
        pbT_free_aa = {0: 0, 1: 0}
        psa3_free = [0]
        pden_free_vv = 0
        stage_free_xdma = {0: None, 1: None}
        Gt_free_mm = {0: 0, 1: 0}
        W4_free_state = {0: 0, 1: 0}
        Sd_free_vv = {0: 0, 1: 0}
        msk_free_mm = {0: 0, 1: 0}
        alf_free = {0: (0, 0), 1: (0, 0)}   # (vv, aa) after scales of that buf
        rden_free_vv = {0: 0, 1: 0}
        rden_ready_aa = {0: 0, 1: 0}
        pden_free_aa = 0
        hnm_free_mm = {0: 0, 1: 0}
        hT_ready_vv = 0
        hT_ready_gg = 0

        REPS = int(os.environ.get("GAT_REPS", "1"))
        AGS = int(os.environ.get("GAT_AGSPLIT", "1"))
        GSPLIT = int(os.environ.get("GAT_GSPLIT", "8"))
        GREP = int(os.environ.get("GAT_GREP", "1"))
        SREP = int(os.environ.get("GAT_SREP", "1"))
        CCREP = int(os.environ.get("GAT_CCREP", "1"))
        NLAYERS = 3 if PHASES >= 4 else min(PHASES, 1)
        do_C = PHASES >= 2
        do_T = PHASES >= 3
        NIDX = KMAX * 128
        NF = [(i * 512, min((i + 1) * 512, NPC)) for i in range(cdiv(NPC, 512))]
        gsz = (TPC + AGS - 1) // AGS
        NCR = cfg.NCORES * 128

        pdma_lin = {0: pdma_loaded}
        ccv_layer = {}
        hT_copies = {}   # per (l, tile): [(sem, val)] per hT chunk after T-copies

        def rec_OUT_piece(pi, f0, f1, l):
            slot = 2 + pi % 2
            pst = pb[slot]
            psl = pst[:, 0:f1 - f0]
            gates = hT_copies[(l, (f1 - 1) // 128)]
            wt("t", "vv", psum_free_vv[slot])
            mmv = None
            for c in range(HCP):
                wt("t", gates[c][0], gates[c][1])
                mmv = op("t", lambda e, psl=psl, c=c, f0=f0, f1=f1:
                         e.matmul(psl, sb_wout[:, c, :], sb_hT[:, c, f0:f1],
                                  start=(c == 0), stop=(c == HCP - 1)),
                         "mm" if c == HCP - 1 else None)
            wt("v", "mm", mmv)
            vvv = op("v", lambda e, psl=psl, f0=f0, f1=f1: e.tensor_scalar(
                sb_osb[:, f0:f1], psl, sb_boutT[:, 0:1], None, ALU.add), "vv")
            psum_free_vv[slot] = vvv

        def rec_lin_reload(l):
            wt("s", "mm", cnt["mm"])
            op("s", lambda e, l=l: e.dma_start(
                sb_lin[:, 0:HCP, :],
                bass.AP(p_lin[l], 0, [[ACOLS, 128], [128 * ACOLS, HCP], [1, ACOLS]])), "pdma", 16)
            pdma_lin[l] = cnt["pdma"]

        def rec_A_bin(l, b, hT_gate, hT_gate_c=None, defer_copies=False):
            inch_p = HIDP if l == 0 else HCP
            wt("t", "pdma", pdma_lin[l])
            if hT_gate:
                wt("t", hT_gate[0], hT_gate[1])
            fch = [(0, 512, pb[2], 2), (512, 1024, pb[3], 3), (1024, ACOLS, ps_a3, -1)]
            mmv = None
            for fi, (f0, f1, pst, slot) in enumerate(fch):
                if slot < 0:
                    wt("t", "vv", psa3_free[0])
                else:
                    wt("t", "vv", psum_free_vv[slot])
                psl = pst[:, 0:f1 - f0]
                for c in range(inch_p):
                    if fi == 0 and hT_gate_c is not None:
                        wt("t", hT_gate_c[c][0], hT_gate_c[c][1])
                    mmv = op("t", lambda e, psl=psl, c=c, b=b, f0=f0, f1=f1, inch_p=inch_p:
                             e.matmul(psl, sb_hT[:, c, b * 128:(b + 1) * 128],
                                      sb_lin[:, c, f0:f1],
                                      start=(c == 0), stop=(c == inch_p - 1)),
                             "mm" if c == inch_p - 1 else None)
            pl = pdma_lin[l]

            def emit_copies():
                sslot = b % 2
                wt("v", "pdma", pl)
                wt("v", "mm", mmv)
                if stage_free_xdma[sslot]:
                    wt("v", *stage_free_xdma[sslot])
                op("v", lambda e, sslot=sslot: e.tensor_copy(sb_stage[:, sslot, 0:512], pb[2][:]))
                op("v", lambda e, sslot=sslot: e.tensor_copy(sb_stage[:, sslot, 512:1024], pb[3][:]))
                vvv = op("v", lambda e, sslot=sslot: e.tensor_copy(
                    sb_stage[:, sslot, HC:HC + 16].bitcast(F32), ps_a3[:]), "vv")
                psum_free_vv[2] = psum_free_vv[3] = psa3_free[0] = vvv
                wt("s", "vv", vvv)
                xsem = "xdA" if sslot == 0 else "xdB"
                xdv = op("s", lambda e, b=b, sslot=sslot:
                         e.dma_start(xh_loc[b * 128:(b + 1) * 128, :], sb_stage[:, sslot, :]),
                         xsem, 16)
                stage_free_xdma[sslot] = (xsem, xdv)
                if (b + 1) % gsz == 0 or b == TPC - 1:
                    b0g = (b // gsz) * gsz
                    nbg = b - b0g + 1
                    wt("g", xsem, xdv)
                    if b > 0 and stage_free_xdma[1 - sslot]:
                        wt("g", *stage_free_xdma[1 - sslot])
                    xf = xh_full2[l % 2]
                    for _cc in range(CCREP):
                        ccv_layer[l] = op("g", lambda e, b0g=b0g, nbg=nbg, xf=xf: e.collective_compute(
                            "AllGather", ALU.bypass,
                            replica_groups=[list(range(cfg.NCORES))],
                            ins=[xh_loc[b0g * 128:(b0g + nbg) * 128, :]],
                            outs=[xf[b0g * NCR:(b0g + nbg) * NCR, :]]), "cc", 1)

            if defer_copies:
                return emit_copies
            emit_copies()
            return None

        for rep in range(REPS):
            if rep > 0:
                wt("s", "mm", cnt["mm"])
                wt("s", "vv", cnt["vv"])
                op("s", lambda e: e.dma_start(
                    sb_lin[:, 0:HIDP, :],
                    bass.AP(p_lin[0], 0, [[ACOLS, 128], [128 * ACOLS, HIDP], [1, ACOLS]])), "pdma", 16)
                pdma_lin[0] = cnt["pdma"]
                wt("t", "pdma", pdma_lin[0])

            # ============ IN-step: hT[:, 0:HIDP, :] = (w_in.T @ in_aug) + b_in ============
            grp = 0
            for oc in range(HIDP):
                for (f0, f1) in NF:
                    slot = grp % 2
                    psl = pb[slot][:, 0:f1 - f0]
                    wt("t", "vv", psum_free_vv[slot])
                    for c in range(DINP):
                        mmv = op("t", lambda e, psl=psl, c=c, oc=oc, f0=f0, f1=f1:
                                 e.matmul(psl, sb_win[:, c, oc * 128:(oc + 1) * 128],
                                          sb_inaug[:, c, f0:f1],
                                          start=(c == 0), stop=(c == DINP - 1)),
                                 "mm" if c == DINP - 1 else None)
                    wt("v", "mm", mmv)
                    vvv = op("v", lambda e, psl=psl, oc=oc, f0=f0, f1=f1:
                             e.tensor_scalar(sb_hT[:, oc, f0:f1], psl,
                                             sb_binT[:, oc:oc + 1], None, ALU.add), "vv")
                    psum_free_vv[slot] = vvv
                    grp += 1
            hT_ready_vv = cnt["vv"]

            # ============ prologue: A(0) + AG(0) ============
            for b in range(TPC):
                rec_A_bin(0, b, ("vv", hT_ready_vv))
            if NLAYERS > 1:
                rec_lin_reload(1)

            # ============ layers (C with interleaved T, A(l+1), AG(l+1)) ============
            for l in range(NLAYERS):
                if not do_C:
                    break
                wt("g", "cc", ccv_layer[l])
                xf = xh_full2[l % 2]
                xh_g = bass.AP(xf, 0, [[RW, NSLOT], [1, RW]])
                xh_sc = bass.AP(xf, HC, [[RW, NSLOT], [1, SCW]])
                gtv = {}
                sdv = {}
                den_mm = {}
                msg_mm = {}
                hnm_ready = {}

                def rec_gather(t, xh_g=None, xh_sc=None):
                    buf = t % 2
                    icol = (t * KMAX) * 8
                    gsm = "gthA" if buf == 0 else "gthC"
                    ssm = "gthB" if buf == 0 else "gthD"
                    wt("g", "mm", Gt_free_mm[buf])
                    wt("g", gsm, gtv.get(t - 2, 0))
                    for _gr in range(GREP):
                        for k0 in range(0, KMAX, GSPLIT):
                            nch = min(GSPLIT, KMAX - k0)
                            gtv[t] = op("g", lambda e, icol=icol, buf=buf, k0=k0, nch=nch, xh_g=xh_g: e.dma_gather(
                                sb_Gt[:, buf, k0:k0 + nch, :], xh_g,
                                sb_isrc[:, icol + k0 * 8:icol + (k0 + nch) * 8],
                                nch * 128, nreg(e, nch * 128), RW, elem_step=RW), gsm, 16)
                    wt("g", "vv", Sd_free_vv[buf])
                    wt("g", ssm, sdv.get(t - 2, 0))
                    for _sr in range(SREP):
                        for k0 in range(0, KMAX, GSPLIT):
                            nch = min(GSPLIT, KMAX - k0)
                            sdv[t] = op("g", lambda e, icol=icol, buf=buf, k0=k0, nch=nch, xh_sc=xh_sc: e.dma_gather(
                                sb_Sd[:, buf, k0:k0 + nch, :], xh_sc,
                                sb_idst[:, icol + k0 * 8:icol + (k0 + nch) * 8],
                                nch * 128, nreg(e, nch * 128), SCW, elem_step=RW), ssm, 16)

                rec_gather(0, xh_g=xh_g, xh_sc=xh_sc)
                rec_gather(1, xh_g=xh_g, xh_sc=xh_sc)

                def rec_tail(tt, l=l):
                    """Epilogue + T + A(l+1)/OUT for tile tt; returns deferred
                    A stage-copy closure (or None)."""
                    buf = tt % 2
                    # --- epilogue (v + a) ---
                    wt("v", "mm", msg_mm[tt])
                    wt("v", "mm", hnm_free_mm[buf])
                    wt("v", "aa", rden_ready_aa[buf])
                    op("v", lambda e: e.drain())
                    vvv = None
                    for h in range(H):
                        psl = pb[h // 2][:, (h % 2) * 256:(h % 2) * 256 + 256]
                        vvv = op("v", lambda e, h=h, buf=buf, psl=psl: e.tensor_scalar(
                            sb_ep1[:, h * 256:(h + 1) * 256], psl,
                            sb_rden[:, buf, h:h + 1], None, ALU.mult),
                            "vv" if h == H - 1 else None)
                    psum_free_vv[0] = psum_free_vv[1] = vvv
                    rden_free_vv[buf] = vvv
                    op("v", lambda e: e.drain())
                    op("v", lambda e, l=l: e.tensor_add(sb_ep2[:], sb_ep1[:], sb_bias[:, l, :]))
                    op("v", lambda e: e.drain())
                    if l < 2:
                        op("v", lambda e: e.tensor_scalar(sb_ep1[:], sb_ep2[:], 0.0, None, ALU.max))
                        vv2 = op("v", lambda e: e.tensor_scalar(sb_ep3[:], sb_ep2[:], 0.0, None, ALU.min), "vv")
                        wt("a", "vv", vv2)
                        aav = op("a", lambda e: e.activation(sb_ep2[:], sb_ep3[:], ACT.Exp), "aa")
                        wt("v", "aa", aav)
                        op("v", lambda e: e.drain())
                        op("v", lambda e: e.tensor_add(sb_ep3[:], sb_ep1[:], sb_ep2[:]))
                        op("v", lambda e: e.drain())
                        vv2 = op("v", lambda e, buf=buf: e.tensor_scalar(
                            sb_hnm[:, buf, :], sb_ep3[:], -1.0, None, ALU.add), "vv")
                    else:
                        vv2 = op("v", lambda e, buf=buf: e.tensor_copy(sb_hnm[:, buf, :], sb_ep2[:]), "vv")
                    hnm_ready[tt] = vv2

                    # --- T-step (t + a copies) ---
                    wt("t", "vv", hnm_ready[tt])
                    last_T = None
                    copies = []
                    for c in range(HCP):
                        sl = c % 2
                        wt("t", "aa", pbT_free_aa[sl])
                        mmv = op("t", lambda e, c=c, sl=sl, buf=buf: e.matmul(
                            pbT[sl][:], sb_hnm[:, buf, c * 128:(c + 1) * 128],
                            sb_ident[:]), "mm")
                        last_T = mmv
                        wt("a", "mm", mmv)
                        aav3 = op("a", lambda e, c=c, sl=sl, tt=tt: e.activation(
                            sb_hT[:, c, tt * 128:(tt + 1) * 128], pbT[sl][:],
                            ACT.Copy), "aa")
                        pbT_free_aa[sl] = aav3
                        copies.append(("aa", aav3))
                    hnm_free_mm[buf] = last_T
                    hT_copies[(l, tt)] = copies

                    deferred = None
                    if l + 1 < NLAYERS:
                        deferred = rec_A_bin(l + 1, tt, None, hT_gate_c=copies,
                                             defer_copies=True)
                        if tt == TPC - 1 and l + 2 < NLAYERS:
                            rec_lin_reload(l + 2)
                    elif l == NLAYERS - 1 and NLAYERS == 3:
                        for pi, (f0, f1) in enumerate(NF):
                            if tt == (f1 - 1) // 128:
                                rec_OUT_piece(pi, f0, f1, l)
                    return deferred

                for t in range(TPC):
                    buf = t % 2
                    # --- masks for this tile (v) ---
                    wt("v", "mm", msk_free_mm[buf])
                    mskv = None
                    for k in range(KMAX):
                        mskv = op("v", lambda e, t=t, k=k, buf=buf: e.tensor_scalar(
                            sb_msk[:, buf, k, :], sb_iota[:],
                            sb_dslot[:, t * KMAX + k:t * KMAX + k + 1], None, ALU.is_equal),
                            "vv" if k == KMAX - 1 else None)
                    msk_ready = mskv
                    # --- score math (v + a) ---
                    wt("v", "gthA" if buf == 0 else "gthC", gtv[t])
                    wt("v", "gthB" if buf == 0 else "gthD", sdv[t])
                    av, aa_ = alf_free[buf]
                    wt("v", "vv", av)
                    wt("v", "aa", aa_)
                    GtF = sb_Gt[:, buf, :, :].bitcast(F32)   # [128, KMAX, RW//2]
                    SdF = sb_Sd[:, buf, :, :].bitcast(F32)   # [128, KMAX, SCW//2]
                    vvv = op("v", lambda e, GtF=GtF, SdF=SdF: e.tensor_add(
                        sb_sc1[:], GtF[:, :, HC // 2:HC // 2 + 4], SdF[:, :, 4:8]), "vv")
                    Sd_free_vv[buf] = vvv
                    wt("a", "vv", vvv)
                    op("a", lambda e: e.activation(sb_sc2[:], sb_sc1[:], ACT.Lrelu,
                                                   alpha=0.2))
                    aav = op("a", lambda e, buf=buf: e.activation(
                        sb_alf[:, buf, :, :], sb_sc2[:], ACT.Exp), "aa")
                    wt("v", "aa", aav)
                    vvv = op("v", lambda e, buf=buf: e.tensor_copy(
                        sb_alb[:, buf, :, :], sb_alf[:, buf, :, :]), "vv")
                    alb_ready = vvv

                    # --- denominator (t) ---
                    wt("t", "vv", alb_ready)
                    wt("t", "vv", msk_ready)
                    wt("t", "aa", pden_free_aa)
                    mmv = None
                    for k in range(KMAX):
                        mmv = op("t", lambda e, k=k, buf=buf: e.matmul(
                            ps_den[:], sb_msk[:, buf, k, :], sb_alb[:, buf, k, :],
                            start=(k == 0), stop=(k == KMAX - 1)),
                            "mm" if k == KMAX - 1 else None)
                    den_mm[t] = mmv

                    # --- deferred tail of previous tile (epi + T + A/OUT) ---
                    if do_T and t > 0:
                        pend_copies = rec_tail(t - 1)
                    else:
                        pend_copies = None

                    # --- alpha-scale G rows in place (v: heads 0-1, a: heads 2-3) ---
                    scale_v = {}
                    scale_a = {}
                    for k in range(KMAX):
                        vvv = None
                        aav2 = None
                        for h in range(2):
                            vvv = op("v", lambda e, k=k, h=h, buf=buf: e.tensor_scalar(
                                sb_Gt[:, buf, k, h * 256:(h + 1) * 256],
                                sb_Gt[:, buf, k, h * 256:(h + 1) * 256],
                                sb_alf[:, buf, k, h:h + 1], None, ALU.mult),
                                "vv" if h == 1 else None)
                        for h in range(2, H):
                            aav2 = op("a", lambda e, k=k, h=h, buf=buf: e.activation(
                                sb_Gt[:, buf, k, h * 256:(h + 1) * 256],
                                sb_Gt[:, buf, k, h * 256:(h + 1) * 256],
                                ACT.Copy, scale=sb_alf[:, buf, k, h:h + 1]),
                                "aa" if h == H - 1 else None)
                        scale_v[k] = vvv
                        scale_a[k] = aav2
                    alf_free[buf] = (scale_v[KMAX - 1], scale_a[KMAX - 1])

                    # --- reciprocal (a), off the v critical path ---
                    wt("a", "mm", den_mm[t])
                    wt("a", "vv", rden_free_vv[buf])
                    aarv = op("a", lambda e, buf=buf: e.activation(
                        sb_rden[:, buf, :], ps_den[:], ACT.Reciprocal,
                        bias=1e-16), "aa")
                    pden_free_aa = aarv
                    rden_ready_aa[buf] = aarv

                    # --- deferred A stage-copies of previous tile (v tail) ---
                    if pend_copies is not None:
                        pend_copies()
                        pend_copies = None

                    # --- message matmuls (t): 2 x 512 cols, accumulate over k ---
                    ch_mm = None
                    for k in range(KMAX):
                        wt("t", "vv", scale_v[k])
                        wt("t", "aa", scale_a[k])
                        if k == 0:
                            wt("t", "vv", psum_free_vv[0])
                            wt("t", "vv", psum_free_vv[1])
                        for half in range(2):
                            ch_mm = op("t", lambda e, k=k, half=half, buf=buf: e.matmul(
                                pb[half][:],
                                sb_msk[:, buf, k, :],
                                sb_Gt[:, buf, k, half * 512:(half + 1) * 512],
                                start=(k == 0), stop=(k == KMAX - 1)),
                                "mm" if half == 1 else None)
                    msg_mm[t] = ch_mm
                    Gt_free_mm[buf] = ch_mm
                    msk_free_mm[buf] = ch_mm
                    if t + 2 < TPC:
                        rec_gather(t + 2, xh_g=xh_g, xh_sc=xh_sc)

                    if not do_T:
                        # --- inline epilogue (ablation path) ---
                        wt("v", "mm", msg_mm[t])
                        op("v", lambda e: e.drain())
                        vvv = None
                        for h in range(H):
                            psl = pb[h // 2][:, (h % 2) * 256:(h % 2) * 256 + 256]
                            vvv = op("v", lambda e, h=h, buf=buf, psl=psl: e.tensor_scalar(
                                sb_ep1[:, h * 256:(h + 1) * 256], psl,
                                sb_rden[:, buf, h:h + 1], None, ALU.mult),
                                "vv" if h == H - 1 else None)
                        psum_free_vv[0] = psum_free_vv[1] = vvv
                        rden_free_vv[buf] = vvv

                # --- flush last tile's tail ---
                if do_T:
                    pend = rec_tail(TPC - 1)
                    if pend is not None:
                        pend()
                if not do_T:
                    break
            hT_ready_vv = cnt["vv"]
            hT_ready_aa = cnt["aa"]

            # ============ OUT-step (fallback when not inlined per tile) ============
            if not (NLAYERS == 3 and do_T):
                grp = 0
                for (f0, f1) in NF:
                    slot = grp % 2
                    psl = pb[slot][:, 0:f1 - f0]
                    wt("t", "vv", psum_free_vv[slot])
                    wt("t", "vv", hT_ready_vv)
                    wt("t", "aa", hT_ready_aa)
                    mmv = None
                    for c in range(HCP):
                        mmv = op("t", lambda e, psl=psl, c=c, f0=f0, f1=f1:
                                 e.matmul(psl, sb_wout[:, c, :], sb_hT[:, c, f0:f1],
                                          start=(c == 0), stop=(c == HCP - 1)),
                                 "mm" if c == HCP - 1 else None)
                    wt("v", "mm", mmv)
                    vvv = op("v", lambda e, psl=psl, f0=f0, f1=f1: e.tensor_scalar(
                        sb_osb[:, f0:f1], psl, sb_boutT[:, 0:1], None, ALU.add), "vv")
                    psum_free_vv[slot] = vvv
                    grp += 1
            wt("s", "vv", cnt["vv"])
            op("s", lambda e: e.dma_start(p_out[:], sb_osb[:]), "xdA", 16)
            wt("s", "xdA", cnt["xdA"])

        # ============ replay ============
        def replay(eng_name):
            def run(e):
                if eng_name == "g":
                    e.load_library(mlp)
                for rec in prog[eng_name]:
                    if rec[0] == "wait":
                        e.wait_ge(sem[rec[1]], rec[2])
                    else:
                        _, fn, inc, amt = rec
                        inst = fn(e)
                        if inc:
                            inst.then_inc(sem[inc], amt)
            return run

        block.gpsimd(replay("g"))
        block.tensor(replay("t"))
        block.vector(replay("v"))
        block.scalar(replay("a"))
        block.sync(replay("s"))

    nc.compile()
    return nc


# =================== host-side data prep ===================

def prep(cfg: Cfg, x, edge_index, node_type, emb_node, w_in, b_in,
         lins, att_ss, att_ds, biases, w_out, b_out):
    """Returns (in_maps, glob) where glob[n] is the packed global row of node n.
    Sets cfg.KMAX. All numpy."""
    N = x.shape[0]
    H, HID, HC, RW, SCW = cfg.H, cfg.HID, cfg.HC, cfg.RW, cfg.SCW
    src = np.concatenate([np.asarray(edge_index[0]), np.arange(N)]).astype(np.int64)
    dst = np.concatenate([np.asarray(edge_index[1]), np.arange(N)]).astype(np.int64)

    deg = np.bincount(dst, minlength=N)
    order = np.argsort(-deg, kind="stable")
    nb = cfg.NBINS
    bin_edges = np.zeros(nb, dtype=np.int64)
    bin_nodes = np.zeros(nb, dtype=np.int64)
    bin_of = np.zeros(N, dtype=np.int64)
    slot_of = np.zeros(N, dtype=np.int64)
    import heapq
    heap = [(0, b) for b in range(nb)]
    heapq.heapify(heap)
    for n in order:
        while True:
            w, b = heapq.heappop(heap)
            if bin_nodes[b] < 128:
                break
        bin_of[n] = b
        slot_of[n] = bin_nodes[b]
        bin_nodes[b] += 1
        bin_edges[b] += deg[n]
        heapq.heappush(heap, (int(bin_edges[b]), b))
    glob = bin_of * 128 + slot_of
    AGS = int(os.environ.get("GAT_AGSPLIT", "1"))
    gsz = (cfg.TPC + AGS - 1) // AGS
    lb = bin_of % cfg.TPC
    grp = lb // gsz
    glob_xh = (grp * gsz * cfg.NCORES * 128 + (bin_of // cfg.TPC) * gsz * 128
               + (lb - grp * gsz) * 128 + slot_of)

    kmax = int(cdiv(int(bin_edges.max()), 128))
    cfg.KMAX = max(kmax, 1)
    KMAX = cfg.KMAX
    TPC, NPC, NSLOT = cfg.TPC, cfg.NPC, cfg.NSLOT

    eb = bin_of[dst]
    eorder = np.argsort(eb, kind="stable")
    es, ed = src[eorder], dst[eorder]
    ebs = eb[eorder]
    starts = np.searchsorted(ebs, np.arange(nb))
    ends = np.searchsorted(ebs, np.arange(nb) + 1)

    CAP = KMAX * 128
    src_g = np.zeros((nb, CAP), dtype=np.int16)
    dst_g = np.zeros((nb, CAP), dtype=np.int16)
    dslot = np.full((nb, CAP), -1, dtype=np.int64)
    for b in range(nb):
        s0, s1 = starts[b], ends[b]
        cntb = s1 - s0
        src_g[b, :cntb] = glob_xh[es[s0:s1]]
        dst_g[b, :cntb] = glob_xh[ed[s0:s1]]
        dslot[b, :cntb] = slot_of[ed[s0:s1]]

    def wrap_idx(flat):
        blk = flat.reshape(TPC * KMAX, 8, 16)
        out = np.zeros((128, TPC * KMAX * 8), dtype=np.int16)
        for gg in range(8):
            out[gg * 16:(gg + 1) * 16, :] = np.transpose(blk, (2, 0, 1)).reshape(16, -1)
        return out

    in_maps = []
    f32 = np.float32
    bf = ml_dtypes.bfloat16

    DIN = cfg.DIN
    DINP = cdiv(DIN, 128)
    HIDP = HID // 128
    HCP = HC // 128
    ACOLS = HC + 8

    X = np.concatenate([np.asarray(x, f32), np.asarray(emb_node, f32)[np.asarray(node_type)]], 1)
    XT = np.zeros((DIN, NSLOT), f32)
    XT[:, glob] = X.T

    lin_augs = []
    for l in range(3):
        lin = np.asarray(lins[l], f32)
        a_sf = np.stack([lin[:, h * HID:(h + 1) * HID] @ np.asarray(att_ss[l], f32)[h] for h in range(H)], 1)
        a_df = np.stack([lin[:, h * HID:(h + 1) * HID] @ np.asarray(att_ds[l], f32)[h] for h in range(H)], 1)
        la = np.concatenate([lin, a_sf, a_df], 1)
        inch_p = HIDP if l == 0 else HCP
        lin_augs.append(la.reshape(inch_p, 128, ACOLS).astype(bf))

    w_in_r = np.asarray(w_in, f32).reshape(DINP, 128, HID).astype(bf)
    b_inT = np.ascontiguousarray(np.asarray(b_in, f32).reshape(HIDP, 128).T)
    bias_bcs = [np.tile(np.asarray(biases[l], f32)[None, :], (128, 1)).astype(f32) for l in range(3)]
    w_out_r = np.asarray(w_out, f32).reshape(HCP, 128, cfg.OUT).astype(bf)
    b_outT = np.asarray(b_out, f32).reshape(cfg.OUT, 1).astype(f32)
    ident = np.eye(128, dtype=bf)
    iota_bc = np.tile(np.arange(128, dtype=bf)[None, :], (128, 1))

    for c in range(cfg.NCORES):
        b0 = c * TPC
        isrc = wrap_idx(src_g[b0:b0 + TPC].reshape(-1))
        idst = wrap_idx(dst_g[b0:b0 + TPC].reshape(-1))
        # dslot column table: [128 partitions(edge slot in chunk), TPC*KMAX]
        ds = dslot[b0:b0 + TPC].reshape(TPC * KMAX, 128)   # [chunk, j]
        dsl = np.ascontiguousarray(ds.T).astype(f32)       # [128, TPC*KMAX]
        in_maps.append({
            "in_augT": XT[:, c * NPC:(c + 1) * NPC].reshape(DINP, 128, NPC).astype(bf),
            "w_in": w_in_r, "b_inT": b_inT,
            "lin0": lin_augs[0], "lin1": lin_augs[1], "lin2": lin_augs[2],
            "bias_bc0": bias_bcs[0], "bias_bc1": bias_bcs[1], "bias_bc2": bias_bcs[2],
            "w_out": w_out_r, "b_outT": b_outT, "ident": ident,
            "idx_src": isrc, "idx_dst": idst,
            "dslot": dsl, "iota_bc": iota_bc,
        })
    return in_maps, glob


def unpack_output(cfg: Cfg, results, glob, N):
    full = np.concatenate([np.asarray(r["outT"]) for r in results], 1)  # [OUT, NSLOT]
    return np.ascontiguousarray(full[:, glob].T.astype(np.float32))


# =================== harness entry point ===================

def kernel(**inputs):
    import numpy as np
    from concourse.bass_utils import run_bass_kernel_spmd

    x = np.asarray(inputs["x"], np.float32)
    N = x.shape[0]
    cfg = Cfg(TPC=10)
    in_maps, glob = prep(
        cfg, x, np.asarray(inputs["edge_index"]), np.asarray(inputs["node_type"]),
        np.asarray(inputs["emb_node"]), np.asarray(inputs["w_in"]), np.asarray(inputs["b_in"]),
        [np.asarray(inputs[f"lin{i}"]) for i in range(3)],
        [np.asarray(inputs[f"att_s{i}"]) for i in range(3)],
        [np.asarray(inputs[f"att_d{i}"]) for i in range(3)],
        [np.asarray(inputs[f"bias{i}"]) for i in range(3)],
        np.asarray(inputs["w_out"]), np.asarray(inputs["b_out"]))
    nc = build_graph(cfg)
    res = run_bass_kernel_spmd(nc, in_maps, core_ids=list(range(cfg.NCORES)))
    return unpack_output(cfg, res.results, glob, N)



# revision 56
# speedup vs baseline: 1.5315x; 1.1584x over previous
"""Distributed 3-layer GAT kernel for TRN2 (8 NeuronCores), v2.

Node layout: nodes greedy-packed into NBINS = NCORES*TPC bins of <=128 slots,
balanced by in-degree. Global row of node n = bin*128 + slot; core c owns bins
[c*TPC,(c+1)*TPC) = rows [c*NPC,(c+1)*NPC).

Per layer:
  A-step : xh[, a_s, a_d] = h @ [lin | att folds]  (TensorE); scores kept as
           raw f32 bytes in bf16 cols [HC, HC+16) via bitcast. DMA to xh_loc,
           grouped AllGather -> xh_full [NSLOT, RW] bf16.
  C-step : per dst tile t (double-buffered): ONE whole-tile dma_gather of src
           rows (RW wide, scores ride along) + ONE dst-score gather (SCW wide).
           Batched score math -> alpha; masks built on-chip via
           is_equal(iota, dslot); denominator via mask lhsT matmuls; alpha
           folded into G rows in-place; 2 message matmuls of 512 cols per
           chunk accumulate in PSUM; epilogue *1/denom, +bias, ELU; T-step
           (transpose to hT) interleaved per tile.
"""
import sys
sys.path.insert(0, "/opt/trn_rl_repo")
import os
from dataclasses import dataclass

import numpy as np
import ml_dtypes

import concourse.bass as bass
import concourse.bacc as bacc
import concourse.mybir as mybir
from concourse.library_config import mlp

BF16 = mybir.dt.bfloat16
F32 = mybir.dt.float32
I16 = mybir.dt.int16
ALU = mybir.AluOpType
ACT = mybir.ActivationFunctionType


@dataclass
class Cfg:
    NCORES: int = 8
    TPC: int = 10
    H: int = 4
    HID: int = 256
    D: int = 384
    OUT: int = 128
    KMAX: int = 17

    @property
    def HC(self):
        return self.H * self.HID

    @property
    def SCW(self):
        return int(__import__('os').environ.get('GAT_SCW', '128'))

    @property
    def RW(self):
        return self.HC + self.SCW

    @property
    def NBINS(self):
        return self.NCORES * self.TPC

    @property
    def NSLOT(self):
        return self.NBINS * 128

    @property
    def NPC(self):
        return self.TPC * 128

    @property
    def DIN(self):
        return self.D + self.HID


def cdiv(a, b):
    return (a + b - 1) // b


def build_graph(cfg: Cfg):
    PHASES = int(os.environ.get("GAT_PHASES", "4"))
    H, HID, HC, RW, SCW = cfg.H, cfg.HID, cfg.HC, cfg.RW, cfg.SCW
    TPC, KMAX, NPC, NSLOT, OUT = cfg.TPC, cfg.KMAX, cfg.NPC, cfg.NSLOT, cfg.OUT
    DINP = cdiv(cfg.DIN, 128)
    HCP = HC // 128
    HIDP = HID // 128
    ACOLS = HC + 8
    assert cfg.DIN % 128 == 0 and HC % 128 == 0 and HID % 128 == 0

    nc = bacc.Bacc("TRN2")

    p_inaug = nc.declare_dram_parameter("in_augT", [DINP, 128, NPC], BF16, isOutput=False)
    p_win = nc.declare_dram_parameter("w_in", [DINP, 128, HID], BF16, isOutput=False)
    p_binT = nc.declare_dram_parameter("b_inT", [128, HIDP], F32, isOutput=False)
    p_lin = [nc.declare_dram_parameter(f"lin{l}", [HIDP if l == 0 else HCP, 128, ACOLS], BF16, isOutput=False) for l in range(3)]
    p_bias = [nc.declare_dram_parameter(f"bias_bc{l}", [128, HC], F32, isOutput=False) for l in range(3)]
    p_wout = nc.declare_dram_parameter("w_out", [HCP, 128, OUT], BF16, isOutput=False)
    p_boutT = nc.declare_dram_parameter("b_outT", [128, 1], F32, isOutput=False)
    p_ident = nc.declare_dram_parameter("ident", [128, 128], BF16, isOutput=False)
    p_isrc = nc.declare_dram_parameter("idx_src", [128, TPC * KMAX * 8], I16, isOutput=False)
    p_idst = nc.declare_dram_parameter("idx_dst", [128, TPC * KMAX * 8], I16, isOutput=False)
    p_dslot = nc.declare_dram_parameter("dslot", [128, TPC * KMAX], F32, isOutput=False)
    p_iota = nc.declare_dram_parameter("iota_bc", [128, 128], BF16, isOutput=False)
    p_out = nc.declare_dram_parameter("outT", [128, NPC], F32, isOutput=True)

    xh_loc = nc.dram_tensor("xh_loc", [NPC, RW], BF16)
    xh_full2 = [nc.dram_tensor(f"xh_full{i}", [NSLOT, RW], BF16, addr_space="Shared")
                for i in range(2)]

    from contextlib import ExitStack
    st = ExitStack()
    with st:
        sb_inaug = st.enter_context(nc.sbuf_tensor("sb_inaug", [128, DINP, NPC], BF16))
        sb_win = st.enter_context(nc.sbuf_tensor("sb_win", [128, DINP, HID], BF16))
        sb_binT = st.enter_context(nc.sbuf_tensor("sb_binT", [128, HIDP], F32))
        sb_lin = st.enter_context(nc.sbuf_tensor("sb_lin", [128, HCP, ACOLS], BF16))
        sb_bias = st.enter_context(nc.sbuf_tensor("sb_bias", [128, 3, HC], F32))
        sb_wout = st.enter_context(nc.sbuf_tensor("sb_wout", [128, HCP, OUT], BF16))
        sb_boutT = st.enter_context(nc.sbuf_tensor("sb_boutT", [128, 1], F32))
        sb_ident = st.enter_context(nc.sbuf_tensor("sb_ident", [128, 128], BF16))
        sb_isrc = st.enter_context(nc.sbuf_tensor("sb_isrc", [128, TPC * KMAX * 8], I16))
        sb_idst = st.enter_context(nc.sbuf_tensor("sb_idst", [128, TPC * KMAX * 8], I16))
        sb_dslot = st.enter_context(nc.sbuf_tensor("sb_dslot", [128, TPC * KMAX], F32))
        sb_iota = st.enter_context(nc.sbuf_tensor("sb_iota", [128, 128], BF16))
        sb_hT = st.enter_context(nc.sbuf_tensor("sb_hT", [128, HCP, NPC], BF16))
        sb_hnm = st.enter_context(nc.sbuf_tensor("sb_hnm", [128, 2, HC], BF16))
        sb_stage = st.enter_context(nc.sbuf_tensor("sb_stage", [128, 2, RW], BF16))
        sb_Gt = st.enter_context(nc.sbuf_tensor("sb_Gt", [128, 2, KMAX, RW], BF16))
        sb_Sd = st.enter_context(nc.sbuf_tensor("sb_Sd", [128, 2, KMAX, SCW], BF16))
        sb_msk = st.enter_context(nc.sbuf_tensor("sb_msk", [128, 2, KMAX, 128], BF16))
        sb_W4 = st.enter_context(nc.sbuf_tensor("sb_W4", [128, 2, 4, 128], BF16))
        sb_alf = st.enter_context(nc.sbuf_tensor("sb_alf", [128, 2, KMAX, 4], F32))
        sb_alb = st.enter_context(nc.sbuf_tensor("sb_alb", [128, 2, KMAX, 4], BF16))
        sb_sc1 = st.enter_context(nc.sbuf_tensor("sb_sc1", [128, KMAX, 4], F32))
        sb_sc2 = st.enter_context(nc.sbuf_tensor("sb_sc2", [128, KMAX, 4], F32))
        sb_sc3 = st.enter_context(nc.sbuf_tensor("sb_sc3", [128, KMAX, 4], F32))
        sb_tmp4 = st.enter_context(nc.sbuf_tensor("sb_tmp4", [128, 4], F32))
        sb_rden = st.enter_context(nc.sbuf_tensor("sb_rden", [128, 2, 4], F32))
        sb_ep1 = st.enter_context(nc.sbuf_tensor("sb_ep1", [128, HC], F32))
        sb_ep2 = st.enter_context(nc.sbuf_tensor("sb_ep2", [128, HC], F32))
        sb_ep3 = st.enter_context(nc.sbuf_tensor("sb_ep3", [128, HC], F32))
        sb_osb = st.enter_context(nc.sbuf_tensor("sb_osb", [128, NPC], F32))
        pb = [st.enter_context(nc.psum_tensor(f"pb{i}", [128, 512], F32)) for i in range(4)]
        pbT = [st.enter_context(nc.psum_tensor(f"pbT{i}", [128, 128], F32)) for i in range(2)]
        ps_a3 = st.enter_context(nc.psum_tensor("ps_a3", [128, 8], F32))
        ps_den = st.enter_context(nc.psum_tensor("ps_den", [128, 4], F32))
        s_pdma = st.enter_context(nc.semaphore("pdma"))
        s_gthA = st.enter_context(nc.semaphore("gthA"))
        s_gthB = st.enter_context(nc.semaphore("gthB"))
        s_gthC = st.enter_context(nc.semaphore("gthC"))
        s_gthD = st.enter_context(nc.semaphore("gthD"))
        s_xdA = st.enter_context(nc.semaphore("xdA"))
        s_xdB = st.enter_context(nc.semaphore("xdB"))
        s_cc = st.enter_context(nc.semaphore("cc"))
        s_mm = st.enter_context(nc.semaphore("mm"))
        s_vv = st.enter_context(nc.semaphore("vv"))
        s_aa = st.enter_context(nc.semaphore("aa"))
        s_gg = st.enter_context(nc.semaphore("gg"))
        block = st.enter_context(nc.Block())
        sem = {"pdma": s_pdma, "gthA": s_gthA, "gthB": s_gthB,
               "gthC": s_gthC, "gthD": s_gthD,
               "xdA": s_xdA, "xdB": s_xdB, "cc": s_cc,
               "mm": s_mm, "vv": s_vv, "aa": s_aa, "gg": s_gg}
        prog = {"g": [], "t": [], "v": [], "a": [], "s": []}
        cnt = {k: 0 for k in sem}
        reg_cache = {}

        def nreg(e, v):
            key = (id(e), v)
            if key not in reg_cache:
                reg_cache[key] = e.to_reg(v)
            return reg_cache[key]

        def op(eng, fn, inc=None, amt=1):
            prog[eng].append(("op", fn, inc, amt))
            if inc:
                cnt[inc] += amt
                return cnt[inc]
            return None

        def wt(eng, sm, val):
            if val and val > 0:
                prog[eng].append(("wait", sm, val))

        # ============ phase 0: loads ============
        loads = [
            (sb_inaug[:], bass.AP(p_inaug, 0, [[NPC, 128], [128 * NPC, DINP], [1, NPC]])),
            (sb_win[:], bass.AP(p_win, 0, [[HID, 128], [128 * HID, DINP], [1, HID]])),
            (sb_binT[:], p_binT[:]),
            (sb_bias[:, 0, :], p_bias[0][:]),
            (sb_bias[:, 1, :], p_bias[1][:]),
            (sb_bias[:, 2, :], p_bias[2][:]),
            (sb_wout[:], bass.AP(p_wout, 0, [[OUT, 128], [128 * OUT, HCP], [1, OUT]])),
            (sb_boutT[:], p_boutT[:]),
            (sb_ident[:], p_ident[:]),
            (sb_isrc[:], p_isrc[:]),
            (sb_idst[:], p_idst[:]),
            (sb_dslot[:], p_dslot[:]),
            (sb_iota[:], p_iota[:]),
            (sb_lin[:, 0:HIDP, :], bass.AP(p_lin[0], 0, [[ACOLS, 128], [128 * ACOLS, HIDP], [1, ACOLS]])),
        ]
        for d, sr in loads:
            op("s", lambda e, d=d, sr=sr: e.dma_start(d, sr), "pdma", 16)
        pdma_loaded = cnt["pdma"]
        if RW > HC + 16:
            op("v", lambda e: e.memset(sb_stage[:, :, HC + 16:RW], 0), "vv")
        if PHASES < 4:
            op("v", lambda e: e.memset(sb_hT[:], 0), "vv")
            op("v", lambda e: e.memset(sb_hnm[:], 0), "vv")
            op("v", lambda e: e.drain())
        for eng in ("g", "t", "v", "a"):
            wt(eng, "pdma", pdma_loaded)

        # persistent cross-step state
        psum_free_vv = {0: 0, 1: 0, 2: 0, 3: 0}   # pb free-after vv
        pbT_free_aa = {0: 0, 1: 0}
        psa3_free = [0]
        pden_free_vv = 0
        stage_free_xdma = {0: None, 1: None}
        Gt_free_mm = {0: 0, 1: 0}
        W4_free_state = {0: 0, 1: 0}
        Sd_free_vv = {0: 0, 1: 0}
        msk_free_mm = {0: 0, 1: 0}
        alf_free = {0: (0, 0), 1: (0, 0)}   # (vv, aa) after scales of that buf
        rden_free_vv = {0: 0, 1: 0}
        hnm_free_mm = {0: 0, 1: 0}
        hT_ready_vv = 0
        hT_ready_gg = 0

        REPS = int(os.environ.get("GAT_REPS", "1"))
        AGS = int(os.environ.get("GAT_AGSPLIT", "1"))
        GSPLIT = int(os.environ.get("GAT_GSPLIT", "8"))
        GREP = int(os.environ.get("GAT_GREP", "1"))
        SREP = int(os.environ.get("GAT_SREP", "1"))
        CCREP = int(os.environ.get("GAT_CCREP", "1"))
        NLAYERS = 3 if PHASES >= 4 else min(PHASES, 1)
        do_C = PHASES >= 2
        do_T = PHASES >= 3
        NIDX = KMAX * 128
        NF = [(i * 512, min((i + 1) * 512, NPC)) for i in range(cdiv(NPC, 512))]
        gsz = (TPC + AGS - 1) // AGS
        NCR = cfg.NCORES * 128

        pdma_lin = {0: pdma_loaded}
        ccv_layer = {}
        hT_copies = {}   # per (l, tile): [(sem, val)] per hT chunk after T-copies

        def rec_OUT_piece(pi, f0, f1, l):
            slot = 2 + pi % 2
            pst = pb[slot]
            psl = pst[:, 0:f1 - f0]
            gates = hT_copies[(l, (f1 - 1) // 128)]
            wt("t", "vv", psum_free_vv[slot])
            mmv = None
            for c in range(HCP):
                wt("t", gates[c][0], gates[c][1])
                mmv = op("t", lambda e, psl=psl, c=c, f0=f0, f1=f1:
                         e.matmul(psl, sb_wout[:, c, :], sb_hT[:, c, f0:f1],
                                  start=(c == 0), stop=(c == HCP - 1)),
                         "mm" if c == HCP - 1 else None)
            wt("v", "mm", mmv)
            vvv = op("v", lambda e, psl=psl, f0=f0, f1=f1: e.tensor_scalar(
                sb_osb[:, f0:f1], psl, sb_boutT[:, 0:1], None, ALU.add), "vv")
            psum_free_vv[slot] = vvv

        def rec_lin_reload(l):
            wt("s", "mm", cnt["mm"])
            op("s", lambda e, l=l: e.dma_start(
                sb_lin[:, 0:HCP, :],
                bass.AP(p_lin[l], 0, [[ACOLS, 128], [128 * ACOLS, HCP], [1, ACOLS]])), "pdma", 16)
            pdma_lin[l] = cnt["pdma"]

        def rec_A_bin(l, b, hT_gate, hT_gate_c=None, defer_copies=False):
            inch_p = HIDP if l == 0 else HCP
            wt("t", "pdma", pdma_lin[l])
            if hT_gate:
                wt("t", hT_gate[0], hT_gate[1])
            fch = [(0, 512, pb[2], 2), (512, 1024, pb[3], 3), (1024, ACOLS, ps_a3, -1)]
            mmv = None
            for fi, (f0, f1, pst, slot) in enumerate(fch):
                if slot < 0:
                    wt("t", "vv", psa3_free[0])
                else:
                    wt("t", "vv", psum_free_vv[slot])
                psl = pst[:, 0:f1 - f0]
                for c in range(inch_p):
                    if fi == 0 and hT_gate_c is not None:
                        wt("t", hT_gate_c[c][0], hT_gate_c[c][1])
                    mmv = op("t", lambda e, psl=psl, c=c, b=b, f0=f0, f1=f1, inch_p=inch_p:
                             e.matmul(psl, sb_hT[:, c, b * 128:(b + 1) * 128],
                                      sb_lin[:, c, f0:f1],
                                      start=(c == 0), stop=(c == inch_p - 1)),
                             "mm" if c == inch_p - 1 else None)
            pl = pdma_lin[l]

            def emit_copies():
                sslot = b % 2
                wt("v", "pdma", pl)
                wt("v", "mm", mmv)
                if stage_free_xdma[sslot]:
                    wt("v", *stage_free_xdma[sslot])
                op("v", lambda e, sslot=sslot: e.tensor_copy(sb_stage[:, sslot, 0:512], pb[2][:]))
                op("v", lambda e, sslot=sslot: e.tensor_copy(sb_stage[:, sslot, 512:1024], pb[3][:]))
                vvv = op("v", lambda e, sslot=sslot: e.tensor_copy(
                    sb_stage[:, sslot, HC:HC + 16].bitcast(F32), ps_a3[:]), "vv")
                psum_free_vv[2] = psum_free_vv[3] = psa3_free[0] = vvv
                wt("s", "vv", vvv)
                xsem = "xdA" if sslot == 0 else "xdB"
                xdv = op("s", lambda e, b=b, sslot=sslot:
                         e.dma_start(xh_loc[b * 128:(b + 1) * 128, :], sb_stage[:, sslot, :]),
                         xsem, 16)
                stage_free_xdma[sslot] = (xsem, xdv)
                if (b + 1) % gsz == 0 or b == TPC - 1:
                    b0g = (b // gsz) * gsz
                    nbg = b - b0g + 1
                    wt("g", xsem, xdv)
                    if b > 0 and stage_free_xdma[1 - sslot]:
                        wt("g", *stage_free_xdma[1 - sslot])
                    xf = xh_full2[l % 2]
                    for _cc in range(CCREP):
                        ccv_layer[l] = op("g", lambda e, b0g=b0g, nbg=nbg, xf=xf: e.collective_compute(
                            "AllGather", ALU.bypass,
                            replica_groups=[list(range(cfg.NCORES))],
                            ins=[xh_loc[b0g * 128:(b0g + nbg) * 128, :]],
                            outs=[xf[b0g * NCR:(b0g + nbg) * NCR, :]]), "cc", 1)

            if defer_copies:
                return emit_copies
            emit_copies()
            return None

        for rep in range(REPS):
            if rep > 0:
                wt("s", "mm", cnt["mm"])
                wt("s", "vv", cnt["vv"])
                op("s", lambda e: e.dma_start(
                    sb_lin[:, 0:HIDP, :],
                    bass.AP(p_lin[0], 0, [[ACOLS, 128], [128 * ACOLS, HIDP], [1, ACOLS]])), "pdma", 16)
                pdma_lin[0] = cnt["pdma"]
                wt("t", "pdma", pdma_lin[0])

            # ============ IN-step: hT[:, 0:HIDP, :] = (w_in.T @ in_aug) + b_in ============
            grp = 0
            for oc in range(HIDP):
                for (f0, f1) in NF:
                    slot = grp % 2
                    psl = pb[slot][:, 0:f1 - f0]
                    wt("t", "vv", psum_free_vv[slot])
                    for c in range(DINP):
                        mmv = op("t", lambda e, psl=psl, c=c, oc=oc, f0=f0, f1=f1:
                                 e.matmul(psl, sb_win[:, c, oc * 128:(oc + 1) * 128],
                                          sb_inaug[:, c, f0:f1],
                                          start=(c == 0), stop=(c == DINP - 1)),
                                 "mm" if c == DINP - 1 else None)
                    wt("v", "mm", mmv)
                    vvv = op("v", lambda e, psl=psl, oc=oc, f0=f0, f1=f1:
                             e.tensor_scalar(sb_hT[:, oc, f0:f1], psl,
                                             sb_binT[:, oc:oc + 1], None, ALU.add), "vv")
                    psum_free_vv[slot] = vvv
                    grp += 1
            hT_ready_vv = cnt["vv"]

            # ============ prologue: A(0) + AG(0) ============
            for b in range(TPC):
                rec_A_bin(0, b, ("vv", hT_ready_vv))
            if NLAYERS > 1:
                rec_lin_reload(1)

            # ============ layers (C with interleaved T, A(l+1), AG(l+1)) ============
            for l in range(NLAYERS):
                if not do_C:
                    break
                wt("g", "cc", ccv_layer[l])
                xf = xh_full2[l % 2]
                xh_g = bass.AP(xf, 0, [[RW, NSLOT], [1, RW]])
                xh_sc = bass.AP(xf, HC, [[RW, NSLOT], [1, SCW]])
                gtv = {}
                sdv = {}
                den_mm = {}
                msg_mm = {}
                hnm_ready = {}

                def rec_gather(t, xh_g=None, xh_sc=None):
                    buf = t % 2
                    icol = (t * KMAX) * 8
                    gsm = "gthA" if buf == 0 else "gthC"
                    ssm = "gthB" if buf == 0 else "gthD"
                    wt("g", "mm", Gt_free_mm[buf])
                    wt("g", gsm, gtv.get(t - 2, 0))
                    for _gr in range(GREP):
                        for k0 in range(0, KMAX, GSPLIT):
                            nch = min(GSPLIT, KMAX - k0)
                            gtv[t] = op("g", lambda e, icol=icol, buf=buf, k0=k0, nch=nch, xh_g=xh_g: e.dma_gather(
                                sb_Gt[:, buf, k0:k0 + nch, :], xh_g,
                                sb_isrc[:, icol + k0 * 8:icol + (k0 + nch) * 8],
                                nch * 128, nreg(e, nch * 128), RW, elem_step=RW), gsm, 16)
                    wt("g", "vv", Sd_free_vv[buf])
                    wt("g", ssm, sdv.get(t - 2, 0))
                    for _sr in range(SREP):
                        for k0 in range(0, KMAX, GSPLIT):
                            nch = min(GSPLIT, KMAX - k0)
                            sdv[t] = op("g", lambda e, icol=icol, buf=buf, k0=k0, nch=nch, xh_sc=xh_sc: e.dma_gather(
                                sb_Sd[:, buf, k0:k0 + nch, :], xh_sc,
                                sb_idst[:, icol + k0 * 8:icol + (k0 + nch) * 8],
                                nch * 128, nreg(e, nch * 128), SCW, elem_step=RW), ssm, 16)

                rec_gather(0, xh_g=xh_g, xh_sc=xh_sc)
                rec_gather(1, xh_g=xh_g, xh_sc=xh_sc)

                def rec_tail(tt, l=l):
                    """Epilogue + T + A(l+1)/OUT for tile tt; returns deferred
                    A stage-copy closure (or None)."""
                    buf = tt % 2
                    # --- epilogue (v + a) ---
                    wt("v", "mm", msg_mm[tt])
                    wt("v", "mm", hnm_free_mm[buf])
                    op("v", lambda e: e.drain())
                    vvv = None
                    for h in range(H):
                        psl = pb[h // 2][:, (h % 2) * 256:(h % 2) * 256 + 256]
                        vvv = op("v", lambda e, h=h, buf=buf, psl=psl: e.tensor_scalar(
                            sb_ep1[:, h * 256:(h + 1) * 256], psl,
                            sb_rden[:, buf, h:h + 1], None, ALU.mult),
                            "vv" if h == H - 1 else None)
                    psum_free_vv[0] = psum_free_vv[1] = vvv
                    rden_free_vv[buf] = vvv
                    op("v", lambda e: e.drain())
                    op("v", lambda e, l=l: e.tensor_add(sb_ep2[:], sb_ep1[:], sb_bias[:, l, :]))
                    op("v", lambda e: e.drain())
                    if l < 2:
                        op("v", lambda e: e.tensor_scalar(sb_ep1[:], sb_ep2[:], 0.0, None, ALU.max))
                        vv2 = op("v", lambda e: e.tensor_scalar(sb_ep3[:], sb_ep2[:], 0.0, None, ALU.min), "vv")
                        wt("a", "vv", vv2)
                        aav = op("a", lambda e: e.activation(sb_ep2[:], sb_ep3[:], ACT.Exp), "aa")
                        wt("v", "aa", aav)
                        op("v", lambda e: e.drain())
                        op("v", lambda e: e.tensor_add(sb_ep3[:], sb_ep1[:], sb_ep2[:]))
                        op("v", lambda e: e.drain())
                        vv2 = op("v", lambda e, buf=buf: e.tensor_scalar(
                            sb_hnm[:, buf, :], sb_ep3[:], -1.0, None, ALU.add), "vv")
                    else:
                        vv2 = op("v", lambda e, buf=buf: e.tensor_copy(sb_hnm[:, buf, :], sb_ep2[:]), "vv")
                    hnm_ready[tt] = vv2

                    # --- T-step (t + a copies) ---
                    wt("t", "vv", hnm_ready[tt])
                    last_T = None
                    copies = []
                    for c in range(HCP):
                        sl = c % 2
                        wt("t", "aa", pbT_free_aa[sl])
                        mmv = op("t", lambda e, c=c, sl=sl, buf=buf: e.matmul(
                            pbT[sl][:], sb_hnm[:, buf, c * 128:(c + 1) * 128],
                            sb_ident[:]), "mm")
                        last_T = mmv
                        wt("a", "mm", mmv)
                        aav3 = op("a", lambda e, c=c, sl=sl, tt=tt: e.activation(
                            sb_hT[:, c, tt * 128:(tt + 1) * 128], pbT[sl][:],
                            ACT.Copy), "aa")
                        pbT_free_aa[sl] = aav3
                        copies.append(("aa", aav3))
                    hnm_free_mm[buf] = last_T
                    hT_copies[(l, tt)] = copies

                    deferred = None
                    if l + 1 < NLAYERS:
                        deferred = rec_A_bin(l + 1, tt, None, hT_gate_c=copies,
                                             defer_copies=True)
                        if tt == TPC - 1 and l + 2 < NLAYERS:
                            rec_lin_reload(l + 2)
                    elif l == NLAYERS - 1 and NLAYERS == 3:
                        for pi, (f0, f1) in enumerate(NF):
                            if tt == (f1 - 1) // 128:
                                rec_OUT_piece(pi, f0, f1, l)
                    return deferred

                for t in range(TPC):
                    buf = t % 2
                    # --- masks for this tile (v) ---
                    wt("v", "mm", msk_free_mm[buf])
                    mskv = None
                    for k in range(KMAX):
                        mskv = op("v", lambda e, t=t, k=k, buf=buf: e.tensor_scalar(
                            sb_msk[:, buf, k, :], sb_iota[:],
                            sb_dslot[:, t * KMAX + k:t * KMAX + k + 1], None, ALU.is_equal),
                            "vv" if k == KMAX - 1 else None)
                    msk_ready = mskv
                    # --- score math (v + a) ---
                    wt("v", "gthA" if buf == 0 else "gthC", gtv[t])
                    wt("v", "gthB" if buf == 0 else "gthD", sdv[t])
                    av, aa_ = alf_free[buf]
                    wt("v", "vv", av)
                    wt("v", "aa", aa_)
                    GtF = sb_Gt[:, buf, :, :].bitcast(F32)   # [128, KMAX, RW//2]
                    SdF = sb_Sd[:, buf, :, :].bitcast(F32)   # [128, KMAX, SCW//2]
                    op("v", lambda e, GtF=GtF, SdF=SdF: e.tensor_add(
                        sb_sc1[:], GtF[:, :, HC // 2:HC // 2 + 4], SdF[:, :, 4:8]))
                    op("v", lambda e: e.drain())
                    op("v", lambda e: e.tensor_scalar(sb_sc2[:], sb_sc1[:], 0.0, None, ALU.max))
                    op("v", lambda e: e.tensor_scalar(sb_sc3[:], sb_sc1[:], 0.0, 0.2, ALU.min, ALU.mult))
                    op("v", lambda e: e.drain())
                    vvv = op("v", lambda e: e.tensor_add(sb_sc1[:], sb_sc2[:], sb_sc3[:]), "vv")
                    wt("a", "vv", vvv)
                    aav = op("a", lambda e, buf=buf: e.activation(
                        sb_alf[:, buf, :, :], sb_sc1[:], ACT.Exp), "aa")
                    wt("v", "aa", aav)
                    vvv = op("v", lambda e, buf=buf: e.tensor_copy(
                        sb_alb[:, buf, :, :], sb_alf[:, buf, :, :]), "vv")
                    alb_ready = vvv
                    Sd_free_vv[buf] = vvv

                    # --- denominator (t) ---
                    wt("t", "vv", alb_ready)
                    wt("t", "vv", msk_ready)
                    wt("t", "vv", pden_free_vv)
                    mmv = None
                    for k in range(KMAX):
                        mmv = op("t", lambda e, k=k, buf=buf: e.matmul(
                            ps_den[:], sb_msk[:, buf, k, :], sb_alb[:, buf, k, :],
                            start=(k == 0), stop=(k == KMAX - 1)),
                            "mm" if k == KMAX - 1 else None)
                    den_mm[t] = mmv

                    # --- deferred tail of previous tile (epi + T + A/OUT) ---
                    if do_T and t > 0:
                        pend_copies = rec_tail(t - 1)
                    else:
                        pend_copies = None

                    # --- alpha-scale G rows in place (v: heads 0-1, a: heads 2-3) ---
                    scale_v = {}
                    scale_a = {}
                    for k in range(KMAX):
                        vvv = None
                        aav2 = None
                        for h in range(2):
                            vvv = op("v", lambda e, k=k, h=h, buf=buf: e.tensor_scalar(
                                sb_Gt[:, buf, k, h * 256:(h + 1) * 256],
                                sb_Gt[:, buf, k, h * 256:(h + 1) * 256],
                                sb_alf[:, buf, k, h:h + 1], None, ALU.mult),
                                "vv" if h == 1 else None)
                        for h in range(2, H):
                            aav2 = op("a", lambda e, k=k, h=h, buf=buf: e.activation(
                                sb_Gt[:, buf, k, h * 256:(h + 1) * 256],
                                sb_Gt[:, buf, k, h * 256:(h + 1) * 256],
                                ACT.Copy, scale=sb_alf[:, buf, k, h:h + 1]),
                                "aa" if h == H - 1 else None)
                        scale_v[k] = vvv
                        scale_a[k] = aav2
                    alf_free[buf] = (scale_v[KMAX - 1], scale_a[KMAX - 1])

                    # --- reciprocal (v), after scales so v doesn't stall on PE ---
                    wt("v", "mm", den_mm[t])
                    wt("v", "vv", rden_free_vv[buf])
                    op("v", lambda e: e.tensor_scalar(sb_tmp4[:], ps_den[:], 1e-16, None, ALU.add))
                    op("v", lambda e: e.drain())
                    vvv = op("v", lambda e, buf=buf: e.reciprocal(sb_rden[:, buf, :], sb_tmp4[:]), "vv")
                    pden_free_vv = vvv

                    # --- deferred A stage-copies of previous tile (v tail) ---
                    if pend_copies is not None:
                        pend_copies()
                        pend_copies = None

                    # --- message matmuls (t): 2 x 512 cols, accumulate over k ---
                    ch_mm = None
                    for k in range(KMAX):
                        wt("t", "vv", scale_v[k])
                        wt("t", "aa", scale_a[k])
                        if k == 0:
                            wt("t", "vv", psum_free_vv[0])
                            wt("t", "vv", psum_free_vv[1])
                        for half in range(2):
                            ch_mm = op("t", lambda e, k=k, half=half, buf=buf: e.matmul(
                                pb[half][:],
                                sb_msk[:, buf, k, :],
                                sb_Gt[:, buf, k, half * 512:(half + 1) * 512],
                                start=(k == 0), stop=(k == KMAX - 1)),
                                "mm" if half == 1 else None)
                    msg_mm[t] = ch_mm
                    Gt_free_mm[buf] = ch_mm
                    msk_free_mm[buf] = ch_mm
                    if t + 2 < TPC:
                        rec_gather(t + 2, xh_g=xh_g, xh_sc=xh_sc)

                    if not do_T:
                        # --- inline epilogue (ablation path) ---
                        wt("v", "mm", msg_mm[t])
                        op("v", lambda e: e.drain())
                        vvv = None
                        for h in range(H):
                            psl = pb[h // 2][:, (h % 2) * 256:(h % 2) * 256 + 256]
                            vvv = op("v", lambda e, h=h, buf=buf, psl=psl: e.tensor_scalar(
                                sb_ep1[:, h * 256:(h + 1) * 256], psl,
                                sb_rden[:, buf, h:h + 1], None, ALU.mult),
                                "vv" if h == H - 1 else None)
                        psum_free_vv[0] = psum_free_vv[1] = vvv
                        rden_free_vv[buf] = vvv

                # --- flush last tile's tail ---
                if do_T:
                    pend = rec_tail(TPC - 1)
                    if pend is not None:
                        pend()
                if not do_T:
                    break
            hT_ready_vv = cnt["vv"]
            hT_ready_aa = cnt["aa"]

            # ============ OUT-step (fallback when not inlined per tile) ============
            if not (NLAYERS == 3 and do_T):
                grp = 0
                for (f0, f1) in NF:
                    slot = grp % 2
                    psl = pb[slot][:, 0:f1 - f0]
                    wt("t", "vv", psum_free_vv[slot])
                    wt("t", "vv", hT_ready_vv)
                    wt("t", "aa", hT_ready_aa)
                    mmv = None
                    for c in range(HCP):
                        mmv = op("t", lambda e, psl=psl, c=c, f0=f0, f1=f1:
                                 e.matmul(psl, sb_wout[:, c, :], sb_hT[:, c, f0:f1],
                                          start=(c == 0), stop=(c == HCP - 1)),
                                 "mm" if c == HCP - 1 else None)
                    wt("v", "mm", mmv)
                    vvv = op("v", lambda e, psl=psl, f0=f0, f1=f1: e.tensor_scalar(
                        sb_osb[:, f0:f1], psl, sb_boutT[:, 0:1], None, ALU.add), "vv")
                    psum_free_vv[slot] = vvv
                    grp += 1
            wt("s", "vv", cnt["vv"])
            op("s", lambda e: e.dma_start(p_out[:], sb_osb[:]), "xdA", 16)
            wt("s", "xdA", cnt["xdA"])

        # ============ replay ============
        def replay(eng_name):
            def run(e):
                if eng_name == "g":
                    e.load_library(mlp)
                for rec in prog[eng_name]:
                    if rec[0] == "wait":
                        e.wait_ge(sem[rec[1]], rec[2])
                    else:
                        _, fn, inc, amt = rec
                        inst = fn(e)
                        if inc:
                            inst.then_inc(sem[inc], amt)
            return run

        block.gpsimd(replay("g"))
        block.tensor(replay("t"))
        block.vector(replay("v"))
        block.scalar(replay("a"))
        block.sync(replay("s"))

    nc.compile()
    return nc


# =================== host-side data prep ===================

def prep(cfg: Cfg, x, edge_index, node_type, emb_node, w_in, b_in,
         lins, att_ss, att_ds, biases, w_out, b_out):
    """Returns (in_maps, glob) where glob[n] is the packed global row of node n.
    Sets cfg.KMAX. All numpy."""
    N = x.shape[0]
    H, HID, HC, RW, SCW = cfg.H, cfg.HID, cfg.HC, cfg.RW, cfg.SCW
    src = np.concatenate([np.asarray(edge_index[0]), np.arange(N)]).astype(np.int64)
    dst = np.concatenate([np.asarray(edge_index[1]), np.arange(N)]).astype(np.int64)

    deg = np.bincount(dst, minlength=N)
    order = np.argsort(-deg, kind="stable")
    nb = cfg.NBINS
    bin_edges = np.zeros(nb, dtype=np.int64)
    bin_nodes = np.zeros(nb, dtype=np.int64)
    bin_of = np.zeros(N, dtype=np.int64)
    slot_of = np.zeros(N, dtype=np.int64)
    import heapq
    heap = [(0, b) for b in range(nb)]
    heapq.heapify(heap)
    for n in order:
        while True:
            w, b = heapq.heappop(heap)
            if bin_nodes[b] < 128:
                break
        bin_of[n] = b
        slot_of[n] = bin_nodes[b]
        bin_nodes[b] += 1
        bin_edges[b] += deg[n]
        heapq.heappush(heap, (int(bin_edges[b]), b))
    glob = bin_of * 128 + slot_of
    AGS = int(os.environ.get("GAT_AGSPLIT", "1"))
    gsz = (cfg.TPC + AGS - 1) // AGS
    lb = bin_of % cfg.TPC
    grp = lb // gsz
    glob_xh = (grp * gsz * cfg.NCORES * 128 + (bin_of // cfg.TPC) * gsz * 128
               + (lb - grp * gsz) * 128 + slot_of)

    kmax = int(cdiv(int(bin_edges.max()), 128))
    cfg.KMAX = max(kmax, 1)
    KMAX = cfg.KMAX
    TPC, NPC, NSLOT = cfg.TPC, cfg.NPC, cfg.NSLOT

    eb = bin_of[dst]
    eorder = np.argsort(eb, kind="stable")
    es, ed = src[eorder], dst[eorder]
    ebs = eb[eorder]
    starts = np.searchsorted(ebs, np.arange(nb))
    ends = np.searchsorted(ebs, np.arange(nb) + 1)

    CAP = KMAX * 128
    src_g = np.zeros((nb, CAP), dtype=np.int16)
    dst_g = np.zeros((nb, CAP), dtype=np.int16)
    dslot = np.full((nb, CAP), -1, dtype=np.int64)
    for b in range(nb):
        s0, s1 = starts[b], ends[b]
        cntb = s1 - s0
        src_g[b, :cntb] = glob_xh[es[s0:s1]]
        dst_g[b, :cntb] = glob_xh[ed[s0:s1]]
        dslot[b, :cntb] = slot_of[ed[s0:s1]]

    def wrap_idx(flat):
        blk = flat.reshape(TPC * KMAX, 8, 16)
        out = np.zeros((128, TPC * KMAX * 8), dtype=np.int16)
        for gg in range(8):
            out[gg * 16:(gg + 1) * 16, :] = np.transpose(blk, (2, 0, 1)).reshape(16, -1)
        return out

    in_maps = []
    f32 = np.float32
    bf = ml_dtypes.bfloat16

    DIN = cfg.DIN
    DINP = cdiv(DIN, 128)
    HIDP = HID // 128
    HCP = HC // 128
    ACOLS = HC + 8

    X = np.concatenate([np.asarray(x, f32), np.asarray(emb_node, f32)[np.asarray(node_type)]], 1)
    XT = np.zeros((DIN, NSLOT), f32)
    XT[:, glob] = X.T

    lin_augs = []
    for l in range(3):
        lin = np.asarray(lins[l], f32)
        a_sf = np.stack([lin[:, h * HID:(h + 1) * HID] @ np.asarray(att_ss[l], f32)[h] for h in range(H)], 1)
        a_df = np.stack([lin[:, h * HID:(h + 1) * HID] @ np.asarray(att_ds[l], f32)[h] for h in range(H)], 1)
        la = np.concatenate([lin, a_sf, a_df], 1)
        inch_p = HIDP if l == 0 else HCP
        lin_augs.append(la.reshape(inch_p, 128, ACOLS).astype(bf))

    w_in_r = np.asarray(w_in, f32).reshape(DINP, 128, HID).astype(bf)
    b_inT = np.ascontiguousarray(np.asarray(b_in, f32).reshape(HIDP, 128).T)
    bias_bcs = [np.tile(np.asarray(biases[l], f32)[None, :], (128, 1)).astype(f32) for l in range(3)]
    w_out_r = np.asarray(w_out, f32).reshape(HCP, 128, cfg.OUT).astype(bf)
    b_outT = np.asarray(b_out, f32).reshape(cfg.OUT, 1).astype(f32)
    ident = np.eye(128, dtype=bf)
    iota_bc = np.tile(np.arange(128, dtype=bf)[None, :], (128, 1))

    for c in range(cfg.NCORES):
        b0 = c * TPC
        isrc = wrap_idx(src_g[b0:b0 + TPC].reshape(-1))
        idst = wrap_idx(dst_g[b0:b0 + TPC].reshape(-1))
        # dslot column table: [128 partitions(edge slot in chunk), TPC*KMAX]
        ds = dslot[b0:b0 + TPC].reshape(TPC * KMAX, 128)   # [chunk, j]
        dsl = np.ascontiguousarray(ds.T).astype(f32)       # [128, TPC*KMAX]
        in_maps.append({
            "in_augT": XT[:, c * NPC:(c + 1) * NPC].reshape(DINP, 128, NPC).astype(bf),
            "w_in": w_in_r, "b_inT": b_inT,
            "lin0": lin_augs[0], "lin1": lin_augs[1], "lin2": lin_augs[2],
            "bias_bc0": bias_bcs[0], "bias_bc1": bias_bcs[1], "bias_bc2": bias_bcs[2],
            "w_out": w_out_r, "b_outT": b_outT, "ident": ident,
            "idx_src": isrc, "idx_dst": idst,
            "dslot": dsl, "iota_bc": iota_bc,
        })
    return in_maps, glob


def unpack_output(cfg: Cfg, results, glob, N):
    full = np.concatenate([np.asarray(r["outT"]) for r in results], 1)  # [OUT, NSLOT]
    return np.ascontiguousarray(full[:, glob].T.astype(np.float32))


# =================== harness entry point ===================

def kernel(**inputs):
    import numpy as np
    from concourse.bass_utils import run_bass_kernel_spmd

    x = np.asarray(inputs["x"], np.float32)
    N = x.shape[0]
    cfg = Cfg(TPC=10)
    in_maps, glob = prep(
        cfg, x, np.asarray(inputs["edge_index"]), np.asarray(inputs["node_type"]),
        np.asarray(inputs["emb_node"]), np.asarray(inputs["w_in"]), np.asarray(inputs["b_in"]),
        [np.asarray(inputs[f"lin{i}"]) for i in range(3)],
        [np.asarray(inputs[f"att_s{i}"]) for i in range(3)],
        [np.asarray(inputs[f"att_d{i}"]) for i in range(3)],
        [np.asarray(inputs[f"bias{i}"]) for i in range(3)],
        np.asarray(inputs["w_out"]), np.asarray(inputs["b_out"]))
    nc = build_graph(cfg)
    res = run_bass_kernel_spmd(nc, in_maps, core_ids=list(range(cfg.NCORES)))
    return unpack_output(cfg, res.results, glob, N)

